# revision 34
# baseline (speedup 1.0000x reference)
"""Trainium2 Bass kernel for nn_DifferentiableADF (angular distribution function).

Computes: for M=500k angle triplets over xyz[8,512,3], the Gaussian-smeared
180-bin histogram of bond angles, normalized to sum 1.

Strategy (8 cores, data-parallel over angles):
  - per-call host->device traffic is minimized (it dominates wall time under
    the axon-proxied PJRT transport): each angle ships as ONE packed int32
    (f | a1<<3 | c<<12 | a2<<21, j-minor slot order = plain reshape on host),
    2MB total across 8 cores, plus a single replicated [3, 4096] raw xyz
    table (48KB). Everything else (index unpack to int16, table replication
    to the 128-partition coordinate-split layout, Gaussian/Hermite derivative
    matrix, acos coefficients, last-chunk validity mask) is computed on
    device each run for ~0 cost.
  - per chunk: DVE unpacks the int32 into 3 int16 table indices; GPSIMD
    ap_gather fetches the 9 coords per angle; a contiguous-block DMA repack
    aligns the stream to compute partitions. Bond vectors + dots on DVE,
    arccos via A&S 4.4.46 polynomial, fast-Gauss-transform moment
    accumulation: theta -> nearest fine bin q (the 180-bin output grid
    itself), moments (1, eps, eps^2, eps^3) scattered into bins via a
    digit-split one-hot matmul on the PE (PSUM accumulates across chunks).
  - AllReduce of the [32,24] moment block, then a tiny matmul against the
    on-device-generated Hermite-derivative matrix reconstructs the exact
    smeared histogram; normalized on device. All cores produce identical
    output; only core 0's shard is fetched.
  - the jitted PJRT callable is built once per process and reused (the
    library helper re-traces/re-lowers per call); warm calls pay only input
    packing (~3ms) + transfer + dispatch + device exec (~0.1ms).

NOTE: f32->i32 tensor_copy on DVE rounds to nearest-even (NOT truncate);
on-device integer digit extraction uses tie-free offsets to get exact floors.
"""

import math
import os
import sys
import time
from contextlib import ExitStack

import numpy as np

sys.path.insert(0, "/opt/trn_rl_repo")

import concourse.bass as bass  # noqa: E402
import concourse.tile as tile  # noqa: E402
from concourse.tile import add_dep_helper  # noqa: E402
from concourse import bacc, mybir  # noqa: E402
from concourse._compat import with_exitstack  # noqa: E402

F32 = mybir.dt.float32
I32 = mybir.dt.int32
I16 = mybir.dt.int16
AF = mybir.ActivationFunctionType
OP = mybir.AluOpType

# ---------------- problem constants ----------------
N_FRAMES = 8
N_ATOMS = 512
N_ANGLES = 500_000
NBINS = 180
H = 180.0 / 179.0  # bin spacing == fine-grid spacing
N_CORES = 8
PER_CORE = N_ANGLES // N_CORES  # 62500
TBL = N_FRAMES * N_ATOMS  # 4096

QL = 8   # low digit of fine-bin index
QH = 24  # high digit (8*24 = 192 >= 180 bins; q in [0,191] all valid rows)
PMOM = 4  # moments kept: eps^0..eps^3
KFLAT = QL * PMOM * QH  # 768 = 6*128
DEG = 180.0 / math.pi

# layout: angle slot s = ((p*CHUNKS + k)*C + j)  p: partition, k: chunk, j: col
CHUNKS = 8
C = 64  # must be multiple of 16 (contiguous-block repack needs 3C % 48 == 0)
SLOTS = 128 * CHUNKS * C  # 65536 >= 62500

# Abramowitz & Stegun 4.4.46: arccos(x) = sqrt(1-x) * sum a_k x^k, x in [0,1]
ACOS_COEF = [
    1.5707963050, -0.2145988016, 0.0889789874, -0.0501743046,
    0.0308918810, -0.0170881256, 0.0066700901, -0.0012624911,
]

_SIM_INIT_RAW = False  # set True (before build) for CoreSim runs only
# True: device returns per-core [32,24] moment blocks, host does the
# reduce + Hermite reconstruction + normalize (no on-device collective).
_HOST_REDUCE = False


def build_amat() -> np.ndarray:
    """A[(ql*PMOM+pm)*QH+qh, b] = g^(pm)(c_q - o_b)/pm!  with g = exp(-x^2/2)."""
    q = np.arange(QL * QH, dtype=np.float64)
    b = np.arange(NBINS, dtype=np.float64)
    d = q[:, None] * H - b[None, :] * H  # [192, 180]
    g0 = np.exp(-0.5 * d * d)
    derivs = [g0, -d * g0, (d * d - 1.0) / 2.0 * g0, (3.0 * d - d**3) / 6.0 * g0]
    a = np.zeros((KFLAT, NBINS), dtype=np.float64)
    for qi in range(QL * QH):
        ql, qh = qi % QL, qi // QL
        for pm in range(PMOM):
            a[(ql * PMOM + pm) * QH + qh, :] = derivs[pm][qi, :]
    return a.astype(np.float32)


@with_exitstack
def adf_kernel(ctx: ExitStack, tc: tile.TileContext, outs, ins, raw, per=None):
    nc = tc.nc
    xyzt_sb, idxs16_raw, gath_raw = raw
    idx_in = ins["idx"]      # [per] int32 packed f|a1<<3|c<<12|a2<<21, j-minor
    xyz3 = ins["xyz3"]       # [3, TBL] f32 raw coordinate-split table
    host_reduce = "outm" in outs
    out = None if host_reduce else outs["out"]  # [180] f32

    chunks, cc = CHUNKS, C

    const_pool = ctx.enter_context(tc.tile_pool(name="const", bufs=1))
    gen_pool = ctx.enter_context(tc.tile_pool(name="gen", bufs=1))
    pool = ctx.enter_context(tc.tile_pool(name="work", bufs=3))
    psum_pool = ctx.enter_context(tc.tile_pool(name="psum", bufs=1, space="PSUM"))
    dram_pool = ctx.enter_context(tc.tile_pool(name="dram", bufs=1, space="DRAM"))

    if _SIM_INIT_RAW:  # CoreSim rejects reads of uninit SBUF; HW tolerates
        for gb in gath_raw:
            nc.vector.memset(gb.ap(), 0.0)

    # ---- replicate the [3, TBL] raw table to partition p = coord p%3 ----
    table_dmas = []
    rep_engines = [nc.sync, nc.scalar]
    for r in range(42):
        td = rep_engines[r % 2].dma_start(
            out=xyzt_sb.ap()[3 * r : 3 * r + 3], in_=xyz3[:]
        )
        table_dmas.append(td)
    td = nc.sync.dma_start(out=xyzt_sb.ap()[126:128], in_=xyz3[0:2])
    table_dmas.append(td)

    # ---- constants ----
    iota_ql = const_pool.tile([128, QL], I32)
    nc.gpsimd.iota(iota_ql[:], pattern=[[1, QL]], base=0, channel_multiplier=0)
    iota_qh = const_pool.tile([128, QH], I32)
    nc.gpsimd.iota(iota_qh[:], pattern=[[1, QH]], base=0, channel_multiplier=0)
    ones_col = const_pool.tile([128, 1], F32)
    nc.vector.memset(ones_col[:], 1.0)
    ones_row = const_pool.tile([1, 128], F32)
    nc.vector.memset(ones_row[:], 1.0)

    # acos coefficients (DEG-scaled, even terms) + tiny-bias constant
    coefs = const_pool.tile([128, 12], F32)
    for col in (0, 2, 4, 6):
        nc.vector.memset(coefs[:, col : col + 1], float(ACOS_COEF[col] * DEG))
    nc.vector.memset(coefs[:, 8:9], 1e-30)

    # ---- last-chunk validity mask, generated on device ----
    # j-minor slot order s' = (k*128 + p)*C + j, valid iff s' < per; math slot
    # (p'=16g+w', col 16jj+w0) has p=16g+w0, j=4w'+jj, so
    # s' - base = p*C + j = (g<<10) + (w0<<6) + (w'<<2) + jj
    maskl = const_pool.tile([128, cc], F32)
    if per is not None:
        m_ci = gen_pool.tile([128, cc], I32)
        nc.gpsimd.iota(m_ci[:], pattern=[[1, cc]], base=0, channel_multiplier=0)
        m_pi = gen_pool.tile([128, 1], I32)
        nc.gpsimd.iota(m_pi[:], pattern=[[1, 1]], base=0, channel_multiplier=1)
        m_a1 = gen_pool.tile([128, 1], I32)
        nc.vector.tensor_scalar(
            out=m_a1[:], in0=m_pi[:], scalar1=15, scalar2=2,
            op0=OP.bitwise_and, op1=OP.logical_shift_left,
        )
        m_a2 = gen_pool.tile([128, 1], I32)
        nc.vector.tensor_scalar(
            out=m_a2[:], in0=m_pi[:], scalar1=4, scalar2=10,
            op0=OP.logical_shift_right, op1=OP.logical_shift_left,
        )
        m_ap = gen_pool.tile([128, 1], I32)
        nc.vector.tensor_tensor(out=m_ap[:], in0=m_a1[:], in1=m_a2[:], op=OP.add)
        m_b1 = gen_pool.tile([128, cc], I32)
        nc.vector.tensor_scalar(
            out=m_b1[:], in0=m_ci[:], scalar1=15, scalar2=6,
            op0=OP.bitwise_and, op1=OP.logical_shift_left,
        )
        m_b2 = gen_pool.tile([128, cc], I32)
        nc.vector.tensor_scalar(
            out=m_b2[:], in0=m_ci[:], scalar1=4, scalar2=None,
            op0=OP.logical_shift_right,
        )
        m_val = gen_pool.tile([128, cc], I32)
        nc.vector.tensor_tensor(out=m_val[:], in0=m_b1[:], in1=m_b2[:], op=OP.add)
        nc.vector.tensor_tensor(
            out=m_val[:], in0=m_val[:], in1=m_ap[:].to_broadcast([128, cc]), op=OP.add
        )
        base = (chunks - 1) * cc * 128
        nc.vector.tensor_scalar(
            out=maskl[:], in0=m_val[:], scalar1=base, scalar2=per,
            op0=OP.add, op1=OP.is_lt,
        )
    else:
        nc.vector.memset(maskl[:], 1.0)

    # ---- generate A on device: a_sb[p, c, b] = A[c*128+p, b] ----
    # flat row r = c*128+p = (ql*PMOM+pm)*QH + qh
    a_sb = const_pool.tile([128, 6, NBINS], F32)
    r_i = gen_pool.tile([128, 6], I32)
    nc.gpsimd.iota(r_i[:], pattern=[[128, 6]], base=0, channel_multiplier=1)
    r_f = gen_pool.tile([128, 6], F32)
    nc.vector.tensor_copy(out=r_f[:], in_=r_i[:])
    # NOTE: f32->i32 tensor_copy rounds to nearest-even. The offsets below are
    # tie-free for the value grids (j/24 resp. j/4), so rint(x - off) == floor(x).
    tmp6 = gen_pool.tile([128, 6], F32)
    nc.vector.tensor_scalar(
        out=tmp6[:], in0=r_f[:], scalar1=float(np.float32(1.0 / 24.0)),
        scalar2=0.47916667, op0=OP.mult, op1=OP.subtract,
    )
    t24i = gen_pool.tile([128, 6], I32)
    nc.vector.tensor_copy(out=t24i[:], in_=tmp6[:])  # rne -> exact floor(r/24)
    t24f = gen_pool.tile([128, 6], F32)
    nc.vector.tensor_copy(out=t24f[:], in_=t24i[:])
    qh_f = gen_pool.tile([128, 6], F32)
    nc.vector.scalar_tensor_tensor(
        out=qh_f[:], in0=t24f[:], scalar=-24.0, in1=r_f[:], op0=OP.mult, op1=OP.add
    )
    nc.vector.tensor_scalar(
        out=tmp6[:], in0=t24f[:], scalar1=0.25, scalar2=0.375,
        op0=OP.mult, op1=OP.subtract,
    )
    qli = gen_pool.tile([128, 6], I32)
    nc.vector.tensor_copy(out=qli[:], in_=tmp6[:])  # rne -> exact floor(t24/4)
    ql_f = gen_pool.tile([128, 6], F32)
    nc.vector.tensor_copy(out=ql_f[:], in_=qli[:])
    pm_f = gen_pool.tile([128, 6], F32)
    nc.vector.scalar_tensor_tensor(
        out=pm_f[:], in0=ql_f[:], scalar=-4.0, in1=t24f[:], op0=OP.mult, op1=OP.add
    )
    qv = gen_pool.tile([128, 6], F32)
    nc.vector.scalar_tensor_tensor(
        out=qv[:], in0=qh_f[:], scalar=float(QL), in1=ql_f[:], op0=OP.mult, op1=OP.add
    )
    # pm one-hot masks
    pmmask = []
    for k in range(PMOM):
        mc = gen_pool.tile([128, 1], F32, tag=f"amc{k}")
        nc.vector.memset(mc[:], float(k))
        mk = gen_pool.tile([128, 6], F32, tag=f"amk{k}")
        nc.vector.tensor_tensor(
            out=mk[:], in0=pm_f[:], in1=mc[:].to_broadcast([128, 6]), op=OP.is_equal
        )
        pmmask.append(mk)
    # b column index
    b_i = gen_pool.tile([128, NBINS], I32)
    nc.gpsimd.iota(b_i[:], pattern=[[1, NBINS]], base=0, channel_multiplier=0)
    b_f = gen_pool.tile([128, NBINS], F32)
    nc.vector.tensor_copy(out=b_f[:], in_=b_i[:])
    # d = (q - b) * H   [128, 6, NBINS]
    d_t = gen_pool.tile([128, 6, NBINS], F32)
    nc.vector.tensor_tensor(
        out=d_t[:],
        in0=qv[:].unsqueeze(2).to_broadcast([128, 6, NBINS]),
        in1=b_f[:].unsqueeze(1).to_broadcast([128, 6, NBINS]),
        op=OP.subtract,
    )
    nc.vector.tensor_scalar(
        out=d_t[:], in0=d_t[:], scalar1=float(np.float32(H)), scalar2=None, op0=OP.mult
    )
    d2_t = gen_pool.tile([128, 6, NBINS], F32)
    nc.vector.tensor_tensor(out=d2_t[:], in0=d_t[:], in1=d_t[:], op=OP.mult)
    g0_t = gen_pool.tile([128, 6, NBINS], F32)
    nc.scalar.activation(g0_t[:], d2_t[:], AF.Exp, scale=-0.5)
    t2_t = gen_pool.tile([128, 6, NBINS], F32)
    nc.vector.tensor_scalar(
        out=t2_t[:], in0=d2_t[:], scalar1=-1.0, scalar2=0.5, op0=OP.add, op1=OP.mult
    )
    u3_t = gen_pool.tile([128, 6, NBINS], F32)
    nc.vector.tensor_scalar(
        out=u3_t[:], in0=d2_t[:], scalar1=-1.0, scalar2=3.0, op0=OP.mult, op1=OP.add
    )
    t3_t = gen_pool.tile([128, 6, NBINS], F32)
    nc.vector.scalar_tensor_tensor(
        out=t3_t[:], in0=d_t[:], scalar=float(np.float32(1.0 / 6.0)), in1=u3_t[:],
        op0=OP.mult, op1=OP.mult,
    )
    acc_t = gen_pool.tile([128, 6, NBINS], F32)
    nc.vector.scalar_tensor_tensor(
        out=acc_t[:], in0=d_t[:], scalar=-1.0,
        in1=pmmask[1][:].unsqueeze(2).to_broadcast([128, 6, NBINS]),
        op0=OP.mult, op1=OP.mult,
    )
    nc.vector.tensor_tensor(
        out=acc_t[:], in0=acc_t[:],
        in1=pmmask[0][:].unsqueeze(2).to_broadcast([128, 6, NBINS]), op=OP.add
    )
    tmp_t = gen_pool.tile([128, 6, NBINS], F32)
    nc.vector.tensor_tensor(
        out=tmp_t[:], in0=t2_t[:],
        in1=pmmask[2][:].unsqueeze(2).to_broadcast([128, 6, NBINS]), op=OP.mult
    )
    nc.vector.tensor_tensor(out=acc_t[:], in0=acc_t[:], in1=tmp_t[:], op=OP.add)
    nc.vector.tensor_tensor(
        out=tmp_t[:], in0=t3_t[:],
        in1=pmmask[3][:].unsqueeze(2).to_broadcast([128, 6, NBINS]), op=OP.mult
    )
    nc.vector.tensor_tensor(out=acc_t[:], in0=acc_t[:], in1=tmp_t[:], op=OP.add)
    nc.vector.tensor_tensor(out=a_sb[:], in0=acc_t[:], in1=g0_t[:], op=OP.mult)
    if "dbg_amat" in outs:
        nc.sync.dma_start(out=outs["dbg_amat"][:], in_=a_sb[:])

    prev_gather = {}  # chunk -> gather inst (ap_gather APs invisible to Tile)
    prev_repack = {}  # chunk -> [repack insts]

    psum_m = psum_pool.tile([QL * PMOM, QH], F32)  # [32, 24] moment accumulator

    def prep_chunk(k):
        # packed int32 indices from DRAM; unpack to 3 int16 table indices.
        # idx is the raw per-core [per] array (no host padding): full chunks
        # slice 8192 ints; the ragged last chunk zero-fills its tail.
        vt = pool.tile([128, cc], I32, tag="vt")
        base = k * 128 * cc
        if per is None or k < chunks - 1:
            nc.sync.dma_start(
                out=vt[:],
                in_=idx_in[base : base + 128 * cc].rearrange("(p j) -> p j", j=cc),
            )
        else:
            rem = per - base
            p_full = rem // cc
            r_rem = rem - p_full * cc
            nc.vector.memset(vt[:], 0)
            nc.sync.dma_start(
                out=vt[0:p_full, :],
                in_=idx_in[base : base + p_full * cc].rearrange("(p j) -> p j", j=cc),
            )
            if r_rem:
                nc.scalar.dma_start(
                    out=vt[p_full : p_full + 1, 0:r_rem],
                    in_=idx_in[base + p_full * cc : base + rem].rearrange(
                        "(p j) -> p j", p=1
                    ),
                )
        fs = pool.tile([128, cc], I32, tag="fs")
        nc.vector.tensor_scalar(
            out=fs[:], in0=vt[:], scalar1=7, scalar2=9,
            op0=OP.bitwise_and, op1=OP.logical_shift_left,
        )
        idxs16 = idxs16_raw[k % 2].ap()
        i16v = idxs16.rearrange("p (j s) -> p j s", s=3)
        idx_copies = []
        at = pool.tile([128, cc], I32, tag="at")
        tt_ = pool.tile([128, cc], I32, tag="tt")
        for si, shift in enumerate((3, 12, 21)):
            nc.vector.tensor_scalar(
                out=at[:], in0=vt[:], scalar1=shift, scalar2=511,
                op0=OP.logical_shift_right, op1=OP.bitwise_and,
            )
            nc.vector.tensor_tensor(out=tt_[:], in0=fs[:], in1=at[:], op=OP.add)
            cp = nc.vector.tensor_copy(out=i16v[:, :, si], in_=tt_[:])
            if k - 2 in prev_gather:  # WAR: slot reuse (2-deep raw buffers)
                add_dep_helper(cp.ins, prev_gather[k - 2].ins, reason="idxs16 WAR")
            idx_copies.append(cp)

        mask = maskl if k == chunks - 1 else None

        # GPSIMD gather: per 16-partition group g the idx stream unwraps as
        # n = m*16 + w (w = source partition%16, m = 3j+s); every partition p
        # of the group gathers the full stream from ITS table row (coord p%3)
        gath = gath_raw[k % 2].ap()
        # j-minor slot order spreads last-chunk pad across partitions, so the
        # full column range must be gathered; pad slots (idx 0) gather finite
        # atom-0 coords and are mask-zeroed downstream.
        ncols = cc
        gth = nc.gpsimd.ap_gather(
            out_ap=gath.unsqueeze(2),
            in_ap=xyzt_sb.ap().unsqueeze(2),
            idxs_ap=idxs16,
            channels=128,
            num_elems=TBL,
            d=1,
            num_idxs=3 * 16 * ncols,
        )
        for tdma in table_dmas:
            add_dep_helper(gth.ins, tdma.ins, reason="gather reads table")
        for cp in idx_copies:
            add_dep_helper(gth.ins, cp.ins, reason="gather reads idxs")
        if k - 2 in prev_repack:  # WAW on gath slot (2-deep raw buffers)
            for rp in prev_repack[k - 2]:
                add_dep_helper(gth.ins, rp.ins, reason="gath WAR vs old repack")
        prev_gather[k] = gth
        return gath, gth, mask

    prepped = {0: prep_chunk(0)}
    for k in range(chunks):
        # issue next chunk's prep + gather BEFORE this chunk's math so the
        # Pool engine (bottleneck) is never starved by DVE trace order
        if k + 1 < chunks:
            prepped[k + 1] = prep_chunk(k + 1)
        gath, gth, mask = prepped.pop(k)

        # contiguous-block repack: math partition p' = 16g + w' takes stream
        # block n in [w'*3cc, (w'+1)*3cc) of its group from rep partition
        # 16g+c. Block = whole triplets since 3cc % 48 == 0. One contiguous
        # DMA per coordinate. In-block: n - w'*3cc = 48*jj + 16*s + w0, the
        # angle being (partition 16g+w0, col 4w'+jj).
        gc = []
        repacks = []
        # three engines: sync/scalar get their own Pool-sem waits; gpsimd
        # follows the gather in Pool program order. (A single engine would
        # leave repacks 2-3 wait-free and racing the gather across queues.)
        rp_engines = [nc.sync, nc.scalar, nc.sync]
        for c3 in range(3):
            gt = pool.tile([128, 3 * cc], F32, tag=f"gc{c3}")
            rp = rp_engines[c3].dma_start(out=gt[:], in_=gath[c3:128:16, :])
            add_dep_helper(rp.ins, gth.ins, reason="repack reads gather output")
            repacks.append(rp)
            gc.append(gt)
        prev_repack[k] = repacks

        if k == 0 and "dbg_g" in outs:
            nc.sync.dma_start(out=outs["dbg_g"][:], in_=gc[0][:])

        # per-(coord, slot) views [128, jj(4), w0(16)] -> 64 angles/partition
        na = cc  # angles per partition per chunk (4*16)
        def sv(ci, si):
            return gc[ci][:].rearrange("p (j s w) -> p j s w", s=3, w=16)[:, :, si, :]

        d11 = pool.tile([128, na], F32, tag="d11")
        d22 = pool.tile([128, na], F32, tag="d22")
        d12 = pool.tile([128, na], F32, tag="d12")
        d11v = d11[:].rearrange("p (j w) -> p j w", w=16)
        d22v = d22[:].rearrange("p (j w) -> p j w", w=16)
        d12v = d12[:].rearrange("p (j w) -> p j w", w=16)
        v1c = pool.tile([128, cc // 16, 16], F32, tag="v1c")
        v2c = pool.tile([128, cc // 16, 16], F32, tag="v2c")
        mm = pool.tile([128, cc // 16, 16], F32, tag="mm")
        for ci in range(3):
            nc.vector.tensor_tensor(out=v1c[:], in0=sv(ci, 0), in1=sv(ci, 1), op=OP.subtract)
            nc.vector.tensor_tensor(out=v2c[:], in0=sv(ci, 2), in1=sv(ci, 1), op=OP.subtract)
            if ci == 0:
                nc.vector.tensor_tensor(out=d11v, in0=v1c[:], in1=v1c[:], op=OP.mult)
                nc.vector.tensor_tensor(out=d22v, in0=v2c[:], in1=v2c[:], op=OP.mult)
                nc.vector.tensor_tensor(out=d12v, in0=v1c[:], in1=v2c[:], op=OP.mult)
            else:
                nc.vector.tensor_tensor(out=mm[:], in0=v1c[:], in1=v1c[:], op=OP.mult)
                nc.vector.tensor_tensor(out=d11v, in0=d11v, in1=mm[:], op=OP.add)
                nc.vector.tensor_tensor(out=mm[:], in0=v2c[:], in1=v2c[:], op=OP.mult)
                nc.vector.tensor_tensor(out=d22v, in0=d22v, in1=mm[:], op=OP.add)
                nc.vector.tensor_tensor(out=mm[:], in0=v1c[:], in1=v2c[:], op=OP.mult)
                nc.vector.tensor_tensor(out=d12v, in0=d12v, in1=mm[:], op=OP.add)

        nn_ = pool.tile([128, cc], F32, tag="nn")
        nc.vector.tensor_tensor(out=nn_[:], in0=d11[:], in1=d22[:], op=OP.mult)
        sq = pool.tile([128, cc], F32, tag="sq")
        # bias keeps padded slots (zero vectors) finite: 1/sqrt(tiny) != inf*0
        nc.scalar.activation(sq[:], nn_[:], AF.Sqrt, bias=coefs[:, 8:9])
        rs = pool.tile([128, cc], F32, tag="rs")
        nc.vector.reciprocal(rs[:], sq[:])
        u = pool.tile([128, cc], F32, tag="u")
        nc.vector.tensor_tensor(out=u[:], in0=d12[:], in1=rs[:], op=OP.mult)
        # clamp |u| <= 1
        au0 = pool.tile([128, cc], F32, tag="au0")
        nc.scalar.activation(au0[:], u[:], AF.Abs)
        au = pool.tile([128, cc], F32, tag="au")
        nc.vector.tensor_scalar(
            out=au[:], in0=au0[:], scalar1=1.0, scalar2=None, op0=OP.min
        )
        sg = pool.tile([128, cc], F32, tag="sg")
        nc.scalar.activation(sg[:], u[:], AF.Sign)

        # theta_abs = sqrt(1-|u|) * P(|u|) in degrees (A&S 4.4.46, 8 terms);
        # theta = 90 + sg*(theta_abs - 90)
        sqterm = pool.tile([128, cc], F32, tag="sqterm")
        nc.scalar.activation(sqterm[:], au[:], AF.Sqrt, bias=1.0, scale=-1.0)
        x2 = pool.tile([128, cc], F32, tag="x2")
        nc.scalar.activation(x2[:], au[:], AF.Square)
        x4 = pool.tile([128, cc], F32, tag="x4")
        nc.scalar.activation(x4[:], x2[:], AF.Square)

        def pair(i_odd, col_even, tag):
            p = pool.tile([128, cc], F32, tag=tag)
            nc.vector.scalar_tensor_tensor(
                out=p[:], in0=au[:], scalar=float(ACOS_COEF[i_odd] * DEG),
                in1=coefs[:, col_even : col_even + 1].to_broadcast([128, cc]),
                op0=OP.mult, op1=OP.add,
            )
            return p

        p01 = pair(1, 0, "p01")
        p23 = pair(3, 2, "p23")
        p45 = pair(5, 4, "p45")
        p67 = pair(7, 6, "p67")
        t1 = pool.tile([128, cc], F32, tag="es1")
        nc.vector.tensor_tensor(out=t1[:], in0=x2[:], in1=p23[:], op=OP.mult)
        nc.vector.tensor_tensor(out=t1[:], in0=t1[:], in1=p01[:], op=OP.add)
        t2 = pool.tile([128, cc], F32, tag="es2")
        nc.vector.tensor_tensor(out=t2[:], in0=x2[:], in1=p67[:], op=OP.mult)
        nc.vector.tensor_tensor(out=t2[:], in0=t2[:], in1=p45[:], op=OP.add)
        nc.vector.tensor_tensor(out=t2[:], in0=t2[:], in1=x4[:], op=OP.mult)
        nc.vector.tensor_tensor(out=t1[:], in0=t1[:], in1=t2[:], op=OP.add)
        thabs = pool.tile([128, cc], F32, tag="thabs")
        nc.vector.tensor_tensor(out=thabs[:], in0=sqterm[:], in1=t1[:], op=OP.mult)
        theta = pool.tile([128, cc], F32, tag="theta")
        nc.vector.tensor_scalar(
            out=theta[:], in0=thabs[:], scalar1=-90.0, scalar2=None, op0=OP.add
        )
        nc.vector.tensor_tensor(out=theta[:], in0=theta[:], in1=sg[:], op=OP.mult)
        nc.vector.tensor_scalar(
            out=theta[:], in0=theta[:], scalar1=90.0, scalar2=None, op0=OP.add
        )
        if k == 0 and "dbg_theta" in outs:
            nc.sync.dma_start(out=outs["dbg_theta"][:], in_=theta[:])

        # fine bin q = round(theta/H) (convert rounding handled by probe: trunc)
        qf_pre = pool.tile([128, cc], F32, tag="qfpre")
        nc.vector.tensor_scalar(
            out=qf_pre[:], in0=theta[:], scalar1=1.0 / H, scalar2=0.5,
            op0=OP.mult, op1=OP.add,
        )
        q_i = pool.tile([128, cc], I32, tag="qi")
        nc.vector.tensor_copy(out=q_i[:], in_=qf_pre[:])
        qf = pool.tile([128, cc], F32, tag="qf")
        nc.vector.tensor_copy(out=qf[:], in_=q_i[:])
        eps = pool.tile([128, cc], F32, tag="eps")
        nc.vector.scalar_tensor_tensor(
            out=eps[:], in0=qf[:], scalar=-H, in1=theta[:], op0=OP.mult, op1=OP.add
        )
        qh_i = pool.tile([128, cc], I32, tag="qhi")
        nc.vector.tensor_scalar(
            out=qh_i[:], in0=q_i[:], scalar1=int(math.log2(QL)), scalar2=None,
            op0=OP.arith_shift_right
        )
        ql_i = pool.tile([128, cc], I32, tag="qli")
        nc.vector.tensor_scalar(
            out=ql_i[:], in0=q_i[:], scalar1=QL - 1, scalar2=None, op0=OP.bitwise_and
        )

        # moment payload E = mask * (1, eps, eps^2, eps^3)
        ee = pool.tile([128, cc, PMOM], F32, tag="ee")
        if mask is None:
            nc.vector.memset(ee[:, :, 0], 1.0)
            nc.vector.tensor_copy(out=ee[:, :, 1], in_=eps[:])
        else:
            nc.vector.tensor_copy(out=ee[:, :, 0], in_=mask[:])
            nc.vector.tensor_tensor(
                out=ee[:, :, 1], in0=eps[:], in1=mask[:], op=OP.mult
            )
        nc.vector.tensor_tensor(
            out=ee[:, :, 2], in0=ee[:, :, 1], in1=eps[:], op=OP.mult
        )
        nc.vector.tensor_tensor(
            out=ee[:, :, 3], in0=ee[:, :, 2], in1=eps[:], op=OP.mult
        )

        # one-hots
        oh_ql = pool.tile([128, cc, QL], F32, tag="ohql")
        nc.vector.tensor_tensor(
            out=oh_ql[:],
            in0=ql_i[:].unsqueeze(2).to_broadcast([128, cc, QL]),
            in1=iota_ql[:].unsqueeze(1).to_broadcast([128, cc, QL]),
            op=OP.is_equal,
        )
        oh_qh = pool.tile([128, cc, QH], F32, tag="ohqh")
        nc.vector.tensor_tensor(
            out=oh_qh[:],
            in0=qh_i[:].unsqueeze(2).to_broadcast([128, cc, QH]),
            in1=iota_qh[:].unsqueeze(1).to_broadcast([128, cc, QH]),
            op=OP.is_equal,
        )
        # lhsT[m, (ql, pm)] = oh_ql[m, ql] * E[m, pm]
        lhs = pool.tile([128, cc, QL * PMOM], F32, tag="lhs")
        nc.vector.tensor_tensor(
            out=lhs[:],
            in0=oh_ql[:].unsqueeze(3).to_broadcast([128, cc, QL, PMOM]),
            in1=ee[:].unsqueeze(2).to_broadcast([128, cc, QL, PMOM]),
            op=OP.mult,
        )

        for j in range(cc):
            nc.tensor.matmul(
                out=psum_m[:],
                lhsT=lhs[:, j, :],
                rhs=oh_qh[:, j, :],
                start=(k == 0 and j == 0),
                stop=(k == chunks - 1 and j == cc - 1),
            )

    # ---- moments out ----
    m_sb = const_pool.tile([QL * PMOM, QH], F32)
    nc.vector.tensor_copy(out=m_sb[:], in_=psum_m[:])
    if host_reduce:
        # ship the tiny per-core moment block; reduce + reconstruct on host
        nc.sync.dma_start(out=outs["outm"][:], in_=m_sb[:])
        return
    m_local = dram_pool.tile([QL * PMOM, QH], F32)
    nc.sync.dma_start(out=m_local[:], in_=m_sb[:])
    m_red = dram_pool.tile([QL * PMOM, QH], F32)
    nc.gpsimd.collective_compute(
        "AllReduce",
        OP.add,
        replica_groups=[list(range(N_CORES))],
        ins=[m_local[:].opt()],
        outs=[m_red[:].opt()],
    )
    # reload flat: element kk = p*QH + n ; rhs chunks [128, 6]
    m_rhs = const_pool.tile([128, 6], F32)
    nc.sync.dma_start(
        out=m_rhs[:], in_=m_red[:].rearrange("p n -> (p n)").rearrange("(c p) -> p c", p=128)
    )

    # ---- final contraction count[b] = sum_k M[k] * A[k, b] ----
    psum_ca = psum_pool.tile([128, 1], F32)
    psum_cb = psum_pool.tile([NBINS - 128, 1], F32)
    for cquad in range(6):
        nc.tensor.matmul(
            out=psum_ca[:], lhsT=a_sb[:, cquad, 0:128], rhs=m_rhs[:, cquad : cquad + 1],
            start=(cquad == 0), stop=(cquad == 5),
        )
    for cquad in range(6):
        nc.tensor.matmul(
            out=psum_cb[:], lhsT=a_sb[:, cquad, 128:NBINS], rhs=m_rhs[:, cquad : cquad + 1],
            start=(cquad == 0), stop=(cquad == 5),
        )
    cnt = const_pool.tile([128, 2], F32)
    nc.vector.memset(cnt[:], 0.0)
    nc.vector.tensor_copy(out=cnt[:, 0:1], in_=psum_ca[:])
    nc.vector.tensor_copy(out=cnt[0 : NBINS - 128, 1:2], in_=psum_cb[:])

    # total + normalize
    psum_t = psum_pool.tile([1, 2], F32)
    nc.tensor.matmul(out=psum_t[:], lhsT=ones_col[:], rhs=cnt[:], start=True, stop=True)
    tt = const_pool.tile([1, 2], F32)
    nc.vector.tensor_copy(out=tt[:], in_=psum_t[:])
    tot = const_pool.tile([1, 1], F32)
    nc.vector.tensor_tensor(out=tot[:], in0=tt[:, 0:1], in1=tt[:, 1:2], op=OP.add)
    rtot = const_pool.tile([1, 1], F32)
    nc.vector.reciprocal(rtot[:], tot[:])
    psum_r = psum_pool.tile([128, 1], F32)
    nc.tensor.matmul(out=psum_r[:], lhsT=ones_row[:], rhs=rtot[:], start=True, stop=True)
    outn = const_pool.tile([128, 2], F32)
    nc.vector.tensor_tensor(
        out=outn[:], in0=cnt[:], in1=psum_r[:].to_broadcast([128, 2]), op=OP.mult
    )
    nc.sync.dma_start(out=out[0:128], in_=outn[:, 0])
    nc.sync.dma_start(out=out[128:NBINS], in_=outn[0 : NBINS - 128, 1])


# ---------------- host side ----------------

def prep_global_inputs(xyz: np.ndarray, angle_list: np.ndarray):
    """Pack host-side directly into the core-concatenated global arrays."""
    flat = np.asarray(xyz, dtype=np.float32).reshape(-1, 3)  # [4096, 3]
    xyz3 = np.ascontiguousarray(flat.T)  # [3, 4096]
    al = np.asarray(angle_list).astype(np.int32, copy=False)
    per = angle_list.shape[0] // N_CORES
    assert per == PER_CORE and per <= SLOTS
    # one int32 per angle: f | a1<<3 | c<<12 | a2<<21. The concatenated
    # global array IS v_all (j-minor slot order s' = (k*128 + p)*C + j per
    # core); the device zero-fills the ragged last chunk, so no host padding
    # or copies are needed.
    v_all = (
        al[:, 0] | (al[:, 1] << 3) | (al[:, 2] << 12) | (al[:, 3] << 21)
    ).astype(np.int32)
    return {"idx": v_all, "xyz3": xyz3}


_PROG_CACHE = {}


def build_program(chunks=CHUNKS, cols=C):
    key = (chunks, cols, _HOST_REDUCE)
    if key in _PROG_CACHE:
        return _PROG_CACHE[key]
    nc = bacc.Bacc("TRN2", target_bir_lowering=False, num_devices=N_CORES)
    idx_len = PER_CORE if chunks == CHUNKS else chunks * 128 * cols
    ins = {
        "idx": nc.dram_tensor("idx", [idx_len], I32, kind="ExternalInput").ap(),
        "xyz3": nc.dram_tensor("xyz3", [3, TBL], F32, kind="ExternalInput").ap(),
    }
    if _HOST_REDUCE:
        outs = {
            "outm": nc.dram_tensor(
                "outm", [QL * PMOM, QH], F32, kind="ExternalOutput"
            ).ap()
        }
    else:
        outs = {"out": nc.dram_tensor("out", [NBINS], F32, kind="ExternalOutput").ap()}
    # raw ap_gather buffers: must be allocated BEFORE TileContext so the tile
    # pools (which claim the free SBUF region at entry) don't overlap them.
    xyzt_sb = nc.alloc_sbuf_tensor("xyzt_sb", [128, TBL], F32)
    idxs16_raw = [
        nc.alloc_sbuf_tensor(f"idxs16r{i}", [128, 3 * cols], I16)
        for i in range(2)
    ]
    gath_raw = [
        nc.alloc_sbuf_tensor(f"gathr{i}", [128, 3 * 16 * cols], F32)
        for i in range(2)
    ]
    raw = (xyzt_sb, idxs16_raw, gath_raw)
    with tile.TileContext(nc) as tc:
        adf_kernel(tc, outs, ins, raw, per=PER_CORE if chunks == CHUNKS else None)
    nc.compile()
    _PROG_CACHE[key] = nc
    return nc


# ---------------- cached PJRT runner ----------------
# run_bass_kernel_spmd rebuilds its jax.jit closure on every call (full
# retrace + lowering each time). Build the jitted sharded callable ONCE and
# reuse it; each call still ships the packed inputs and runs the device.

_RUNNER = None


def _get_runner():
    global _RUNNER
    if _RUNNER is not None:
        return _RUNNER
    import jax
    from jax.sharding import Mesh, PartitionSpec
    from jax.experimental.shard_map import shard_map
    from concourse.bass2jax import (
        _bass_exec_p, install_neuronx_cc_hook, partition_id_tensor,
    )

    nc = build_program()
    install_neuronx_cc_hook()

    partition_name = nc.partition_id_tensor.name if nc.partition_id_tensor else None
    in_names, out_names, out_avals = [], [], []
    for alloc in nc.m.functions[0].allocations:
        if not isinstance(alloc, mybir.MemoryLocationSet):
            continue
        name = alloc.memorylocations[0].name
        if alloc.kind == "ExternalInput":
            if name != partition_name:
                in_names.append(name)
        elif alloc.kind == "ExternalOutput":
            shape = tuple(alloc.tensor_shape)
            dtype = mybir.dt.np(alloc.dtype)
            out_names.append(name)
            out_avals.append(jax.core.ShapedArray(shape, dtype))
    assert nc.dbg_addr is None, "debug build unsupported in cached PJRT runner"
    # the NKI lowering only consumes ExternalInput-named operands and the
    # alias map is empty, so no donated zero-output operands are needed; the
    # kernel writes every element of 'out'.
    in_names_full = in_names + (
        [partition_name] if partition_name is not None else []
    )

    def _body(*args):
        operands = list(args)
        if partition_name is not None:
            operands.append(partition_id_tensor())
        outs_ = _bass_exec_p.bind(
            *operands,
            out_avals=tuple(out_avals),
            in_names=tuple(in_names_full),
            out_names=tuple(out_names),
            lowering_input_output_aliases=(),
            sim_require_finite=True,
            sim_require_nnan=True,
            nc=nc,
        )
        return tuple(outs_)

    devices = jax.devices()[:N_CORES]
    assert len(devices) == N_CORES
    mesh = Mesh(np.asarray(devices), ("core",))
    # xyz3 is identical on every core: ship one copy, replicated in_spec
    in_specs = tuple(
        PartitionSpec() if n == "xyz3" else PartitionSpec("core") for n in in_names
    )
    sharded = jax.jit(
        shard_map(
            _body, mesh=mesh,
            in_specs=in_specs,
            out_specs=(PartitionSpec("core"),) * len(out_names),
            check_rep=False,
        ),
        keep_unused=True,
    )

    out_pos = {name: i for i, name in enumerate(out_names)}

    def run(global_map):
        concat_in = [global_map[name] for name in in_names]
        out_arrs = sharded(*concat_in)
        if "outm" in out_pos:
            # per-core moment blocks: fetch all 8 shards, reduce on host
            return np.asarray(out_arrs[out_pos["outm"]])
        # fetch only core 0's shard of 'out' (all cores produce identical output)
        shard = out_arrs[out_pos["out"]].addressable_shards[0].data
        return np.asarray(shard)

    _RUNNER = run
    return run


_AMAT = None


def kernel(**inputs) -> np.ndarray:
    global _AMAT
    xyz = np.asarray(inputs["xyz"], dtype=np.float32)
    angle_list = np.asarray(inputs["angle_list"])
    run = _get_runner()  # one-time program build + jit construction
    if _HOST_REDUCE and _AMAT is None:
        _AMAT = build_amat().astype(np.float64)
    # timed region matches the baseline definition: device dispatch + transfer
    # + exec + result fetch (host packing excluded, as in the original)
    gm = prep_global_inputs(xyz, angle_list)
    t0 = time.time()
    out = run(gm)
    if _HOST_REDUCE:
        m = out.reshape(N_CORES, QL * PMOM * QH).sum(axis=0, dtype=np.float64)
        count = m @ _AMAT
        out = (count / count.sum()).astype(np.float32)
    kernel._last_run_s = time.time() - t0
    kernel._last_results = None
    return np.asarray(out, dtype=np.float32)


if __name__ == "__main__":
    # smoke: build only
    build_program()
    print("program built ok")


# revision 35
# speedup vs baseline: 1.0314x; 1.0314x over previous
"""Trainium2 Bass kernel for nn_DifferentiableADF (angular distribution function).

Computes: for M=500k angle triplets over xyz[8,512,3], the Gaussian-smeared
180-bin histogram of bond angles, normalized to sum 1.

Strategy (8 cores, data-parallel over angles):
  - per-call host->device traffic is minimized (it dominates wall time under
    the axon-proxied PJRT transport: ~70ms fixed RPC legs + ~12ms/MB): each
    angle ships as ONE packed int32 (f | a1<<3 | c<<12 | a2<<21, j-minor slot
    order), and the concatenated global array is exactly the packed [500000]
    vector — no host padding or copies; the device zero-fills the ragged last
    chunk. Plus one replicated [3, 4096] raw xyz table (48KB). Everything
    else (index unpack to int16, table replication to the 128-partition
    coordinate-split layout, Gaussian/Hermite derivative matrix, acos
    coefficients, last-chunk validity mask) is computed on device each run
    for ~0 cost. Total shipped: 2.0MB, the entropy of the angle data.
  - per chunk: DVE unpacks the int32 into 3 int16 table indices; GPSIMD
    ap_gather fetches the 9 coords per angle; a contiguous-block DMA repack
    aligns the stream to compute partitions. Bond vectors + dots on DVE,
    arccos via A&S 4.4.46 polynomial, fast-Gauss-transform moment
    accumulation: theta -> nearest fine bin q (the 180-bin output grid
    itself), moments (1, eps, eps^2, eps^3) scattered into bins via a
    digit-split one-hot matmul on the PE (PSUM accumulates across chunks).
  - AllReduce of the [32,24] moment block, then a tiny matmul against the
    on-device-generated Hermite-derivative matrix reconstructs the exact
    smeared histogram; normalized on device. All cores produce identical
    output; only core 0's shard is fetched.
  - the jitted PJRT callable is built once per process and reused (the
    library helper re-traces/re-lowers per call); warm calls pay only input
    packing (~3ms) + transfer + dispatch + device exec (~0.1ms).

NOTE: f32->i32 tensor_copy on DVE rounds to nearest-even (NOT truncate);
on-device integer digit extraction uses tie-free offsets to get exact floors.
"""

import math
import os
import sys
import time
from contextlib import ExitStack

import numpy as np

sys.path.insert(0, "/opt/trn_rl_repo")

import concourse.bass as bass  # noqa: E402
import concourse.tile as tile  # noqa: E402
from concourse.tile import add_dep_helper  # noqa: E402
from concourse import bacc, mybir  # noqa: E402
from concourse._compat import with_exitstack  # noqa: E402

F32 = mybir.dt.float32
I32 = mybir.dt.int32
I16 = mybir.dt.int16
AF = mybir.ActivationFunctionType
OP = mybir.AluOpType

# ---------------- problem constants ----------------
N_FRAMES = 8
N_ATOMS = 512
N_ANGLES = 500_000
NBINS = 180
H = 180.0 / 179.0  # bin spacing == fine-grid spacing
N_CORES = 8
PER_CORE = N_ANGLES // N_CORES  # 62500
TBL = N_FRAMES * N_ATOMS  # 4096

QL = 8   # low digit of fine-bin index
QH = 24  # high digit (8*24 = 192 >= 180 bins; q in [0,191] all valid rows)
PMOM = 4  # moments kept: eps^0..eps^3
KFLAT = QL * PMOM * QH  # 768 = 6*128
DEG = 180.0 / math.pi

# layout: angle slot s = ((p*CHUNKS + k)*C + j)  p: partition, k: chunk, j: col
CHUNKS = 8
C = 64  # must be multiple of 16 (contiguous-block repack needs 3C % 48 == 0)
SLOTS = 128 * CHUNKS * C  # 65536 >= 62500

# Abramowitz & Stegun 4.4.46: arccos(x) = sqrt(1-x) * sum a_k x^k, x in [0,1]
ACOS_COEF = [
    1.5707963050, -0.2145988016, 0.0889789874, -0.0501743046,
    0.0308918810, -0.0170881256, 0.0066700901, -0.0012624911,
]

_SIM_INIT_RAW = False  # set True (before build) for CoreSim runs only
# True: device returns per-core [32,24] moment blocks, host does the
# reduce + Hermite reconstruction + normalize (no on-device collective).
_HOST_REDUCE = False


def build_amat() -> np.ndarray:
    """A[(ql*PMOM+pm)*QH+qh, b] = g^(pm)(c_q - o_b)/pm!  with g = exp(-x^2/2)."""
    q = np.arange(QL * QH, dtype=np.float64)
    b = np.arange(NBINS, dtype=np.float64)
    d = q[:, None] * H - b[None, :] * H  # [192, 180]
    g0 = np.exp(-0.5 * d * d)
    derivs = [g0, -d * g0, (d * d - 1.0) / 2.0 * g0, (3.0 * d - d**3) / 6.0 * g0]
    a = np.zeros((KFLAT, NBINS), dtype=np.float64)
    for qi in range(QL * QH):
        ql, qh = qi % QL, qi // QL
        for pm in range(PMOM):
            a[(ql * PMOM + pm) * QH + qh, :] = derivs[pm][qi, :]
    return a.astype(np.float32)


@with_exitstack
def adf_kernel(ctx: ExitStack, tc: tile.TileContext, outs, ins, raw, per=None):
    nc = tc.nc
    xyzt_sb, idxs16_raw, gath_raw = raw
    idx_in = ins["idx"]      # [per] int32 packed f|a1<<3|c<<12|a2<<21, j-minor
    xyz3 = ins["xyz3"]       # [3, TBL] f32 raw coordinate-split table
    host_reduce = "outm" in outs
    out = None if host_reduce else outs["out"]  # [180] f32

    chunks, cc = CHUNKS, C

    const_pool = ctx.enter_context(tc.tile_pool(name="const", bufs=1))
    gen_pool = ctx.enter_context(tc.tile_pool(name="gen", bufs=1))
    pool = ctx.enter_context(tc.tile_pool(name="work", bufs=3))
    psum_pool = ctx.enter_context(tc.tile_pool(name="psum", bufs=1, space="PSUM"))
    dram_pool = ctx.enter_context(tc.tile_pool(name="dram", bufs=1, space="DRAM"))

    if _SIM_INIT_RAW:  # CoreSim rejects reads of uninit SBUF; HW tolerates
        for gb in gath_raw:
            nc.vector.memset(gb.ap(), 0.0)

    # ---- replicate the [3, TBL] raw table to partition p = coord p%3 ----
    table_dmas = []
    rep_engines = [nc.sync, nc.scalar]
    for r in range(42):
        td = rep_engines[r % 2].dma_start(
            out=xyzt_sb.ap()[3 * r : 3 * r + 3], in_=xyz3[:]
        )
        table_dmas.append(td)
    td = nc.sync.dma_start(out=xyzt_sb.ap()[126:128], in_=xyz3[0:2])
    table_dmas.append(td)

    # ---- constants ----
    iota_ql = const_pool.tile([128, QL], I32)
    nc.gpsimd.iota(iota_ql[:], pattern=[[1, QL]], base=0, channel_multiplier=0)
    iota_qh = const_pool.tile([128, QH], I32)
    nc.gpsimd.iota(iota_qh[:], pattern=[[1, QH]], base=0, channel_multiplier=0)
    ones_col = const_pool.tile([128, 1], F32)
    nc.vector.memset(ones_col[:], 1.0)
    ones_row = const_pool.tile([1, 128], F32)
    nc.vector.memset(ones_row[:], 1.0)

    # acos coefficients (DEG-scaled, even terms) + tiny-bias constant
    coefs = const_pool.tile([128, 12], F32)
    for col in (0, 2, 4, 6):
        nc.vector.memset(coefs[:, col : col + 1], float(ACOS_COEF[col] * DEG))
    nc.vector.memset(coefs[:, 8:9], 1e-30)

    # ---- last-chunk validity mask, generated on device ----
    # j-minor slot order s' = (k*128 + p)*C + j, valid iff s' < per; math slot
    # (p'=16g+w', col 16jj+w0) has p=16g+w0, j=4w'+jj, so
    # s' - base = p*C + j = (g<<10) + (w0<<6) + (w'<<2) + jj
    maskl = const_pool.tile([128, cc], F32)
    if per is not None:
        m_ci = gen_pool.tile([128, cc], I32)
        nc.gpsimd.iota(m_ci[:], pattern=[[1, cc]], base=0, channel_multiplier=0)
        m_pi = gen_pool.tile([128, 1], I32)
        nc.gpsimd.iota(m_pi[:], pattern=[[1, 1]], base=0, channel_multiplier=1)
        m_a1 = gen_pool.tile([128, 1], I32)
        nc.vector.tensor_scalar(
            out=m_a1[:], in0=m_pi[:], scalar1=15, scalar2=2,
            op0=OP.bitwise_and, op1=OP.logical_shift_left,
        )
        m_a2 = gen_pool.tile([128, 1], I32)
        nc.vector.tensor_scalar(
            out=m_a2[:], in0=m_pi[:], scalar1=4, scalar2=10,
            op0=OP.logical_shift_right, op1=OP.logical_shift_left,
        )
        m_ap = gen_pool.tile([128, 1], I32)
        nc.vector.tensor_tensor(out=m_ap[:], in0=m_a1[:], in1=m_a2[:], op=OP.add)
        m_b1 = gen_pool.tile([128, cc], I32)
        nc.vector.tensor_scalar(
            out=m_b1[:], in0=m_ci[:], scalar1=15, scalar2=6,
            op0=OP.bitwise_and, op1=OP.logical_shift_left,
        )
        m_b2 = gen_pool.tile([128, cc], I32)
        nc.vector.tensor_scalar(
            out=m_b2[:], in0=m_ci[:], scalar1=4, scalar2=None,
            op0=OP.logical_shift_right,
        )
        m_val = gen_pool.tile([128, cc], I32)
        nc.vector.tensor_tensor(out=m_val[:], in0=m_b1[:], in1=m_b2[:], op=OP.add)
        nc.vector.tensor_tensor(
            out=m_val[:], in0=m_val[:], in1=m_ap[:].to_broadcast([128, cc]), op=OP.add
        )
        base = (chunks - 1) * cc * 128
        nc.vector.tensor_scalar(
            out=maskl[:], in0=m_val[:], scalar1=base, scalar2=per,
            op0=OP.add, op1=OP.is_lt,
        )
    else:
        nc.vector.memset(maskl[:], 1.0)

    # ---- generate A on device: a_sb[p, c, b] = A[c*128+p, b] ----
    # flat row r = c*128+p = (ql*PMOM+pm)*QH + qh
    a_sb = const_pool.tile([128, 6, NBINS], F32)
    r_i = gen_pool.tile([128, 6], I32)
    nc.gpsimd.iota(r_i[:], pattern=[[128, 6]], base=0, channel_multiplier=1)
    r_f = gen_pool.tile([128, 6], F32)
    nc.vector.tensor_copy(out=r_f[:], in_=r_i[:])
    # NOTE: f32->i32 tensor_copy rounds to nearest-even. The offsets below are
    # tie-free for the value grids (j/24 resp. j/4), so rint(x - off) == floor(x).
    tmp6 = gen_pool.tile([128, 6], F32)
    nc.vector.tensor_scalar(
        out=tmp6[:], in0=r_f[:], scalar1=float(np.float32(1.0 / 24.0)),
        scalar2=0.47916667, op0=OP.mult, op1=OP.subtract,
    )
    t24i = gen_pool.tile([128, 6], I32)
    nc.vector.tensor_copy(out=t24i[:], in_=tmp6[:])  # rne -> exact floor(r/24)
    t24f = gen_pool.tile([128, 6], F32)
    nc.vector.tensor_copy(out=t24f[:], in_=t24i[:])
    qh_f = gen_pool.tile([128, 6], F32)
    nc.vector.scalar_tensor_tensor(
        out=qh_f[:], in0=t24f[:], scalar=-24.0, in1=r_f[:], op0=OP.mult, op1=OP.add
    )
    nc.vector.tensor_scalar(
        out=tmp6[:], in0=t24f[:], scalar1=0.25, scalar2=0.375,
        op0=OP.mult, op1=OP.subtract,
    )
    qli = gen_pool.tile([128, 6], I32)
    nc.vector.tensor_copy(out=qli[:], in_=tmp6[:])  # rne -> exact floor(t24/4)
    ql_f = gen_pool.tile([128, 6], F32)
    nc.vector.tensor_copy(out=ql_f[:], in_=qli[:])
    pm_f = gen_pool.tile([128, 6], F32)
    nc.vector.scalar_tensor_tensor(
        out=pm_f[:], in0=ql_f[:], scalar=-4.0, in1=t24f[:], op0=OP.mult, op1=OP.add
    )
    qv = gen_pool.tile([128, 6], F32)
    nc.vector.scalar_tensor_tensor(
        out=qv[:], in0=qh_f[:], scalar=float(QL), in1=ql_f[:], op0=OP.mult, op1=OP.add
    )
    # pm one-hot masks
    pmmask = []
    for k in range(PMOM):
        mc = gen_pool.tile([128, 1], F32, tag=f"amc{k}")
        nc.vector.memset(mc[:], float(k))
        mk = gen_pool.tile([128, 6], F32, tag=f"amk{k}")
        nc.vector.tensor_tensor(
            out=mk[:], in0=pm_f[:], in1=mc[:].to_broadcast([128, 6]), op=OP.is_equal
        )
        pmmask.append(mk)
    # b column index
    b_i = gen_pool.tile([128, NBINS], I32)
    nc.gpsimd.iota(b_i[:], pattern=[[1, NBINS]], base=0, channel_multiplier=0)
    b_f = gen_pool.tile([128, NBINS], F32)
    nc.vector.tensor_copy(out=b_f[:], in_=b_i[:])
    # d = (q - b) * H   [128, 6, NBINS]
    d_t = gen_pool.tile([128, 6, NBINS], F32)
    nc.vector.tensor_tensor(
        out=d_t[:],
        in0=qv[:].unsqueeze(2).to_broadcast([128, 6, NBINS]),
        in1=b_f[:].unsqueeze(1).to_broadcast([128, 6, NBINS]),
        op=OP.subtract,
    )
    nc.vector.tensor_scalar(
        out=d_t[:], in0=d_t[:], scalar1=float(np.float32(H)), scalar2=None, op0=OP.mult
    )
    d2_t = gen_pool.tile([128, 6, NBINS], F32)
    nc.vector.tensor_tensor(out=d2_t[:], in0=d_t[:], in1=d_t[:], op=OP.mult)
    g0_t = gen_pool.tile([128, 6, NBINS], F32)
    nc.scalar.activation(g0_t[:], d2_t[:], AF.Exp, scale=-0.5)
    t2_t = gen_pool.tile([128, 6, NBINS], F32)
    nc.vector.tensor_scalar(
        out=t2_t[:], in0=d2_t[:], scalar1=-1.0, scalar2=0.5, op0=OP.add, op1=OP.mult
    )
    u3_t = gen_pool.tile([128, 6, NBINS], F32)
    nc.vector.tensor_scalar(
        out=u3_t[:], in0=d2_t[:], scalar1=-1.0, scalar2=3.0, op0=OP.mult, op1=OP.add
    )
    t3_t = gen_pool.tile([128, 6, NBINS], F32)
    nc.vector.scalar_tensor_tensor(
        out=t3_t[:], in0=d_t[:], scalar=float(np.float32(1.0 / 6.0)), in1=u3_t[:],
        op0=OP.mult, op1=OP.mult,
    )
    acc_t = gen_pool.tile([128, 6, NBINS], F32)
    nc.vector.scalar_tensor_tensor(
        out=acc_t[:], in0=d_t[:], scalar=-1.0,
        in1=pmmask[1][:].unsqueeze(2).to_broadcast([128, 6, NBINS]),
        op0=OP.mult, op1=OP.mult,
    )
    nc.vector.tensor_tensor(
        out=acc_t[:], in0=acc_t[:],
        in1=pmmask[0][:].unsqueeze(2).to_broadcast([128, 6, NBINS]), op=OP.add
    )
    tmp_t = gen_pool.tile([128, 6, NBINS], F32)
    nc.vector.tensor_tensor(
        out=tmp_t[:], in0=t2_t[:],
        in1=pmmask[2][:].unsqueeze(2).to_broadcast([128, 6, NBINS]), op=OP.mult
    )
    nc.vector.tensor_tensor(out=acc_t[:], in0=acc_t[:], in1=tmp_t[:], op=OP.add)
    nc.vector.tensor_tensor(
        out=tmp_t[:], in0=t3_t[:],
        in1=pmmask[3][:].unsqueeze(2).to_broadcast([128, 6, NBINS]), op=OP.mult
    )
    nc.vector.tensor_tensor(out=acc_t[:], in0=acc_t[:], in1=tmp_t[:], op=OP.add)
    nc.vector.tensor_tensor(out=a_sb[:], in0=acc_t[:], in1=g0_t[:], op=OP.mult)
    if "dbg_amat" in outs:
        nc.sync.dma_start(out=outs["dbg_amat"][:], in_=a_sb[:])

    prev_gather = {}  # chunk -> gather inst (ap_gather APs invisible to Tile)
    prev_repack = {}  # chunk -> [repack insts]

    psum_m = psum_pool.tile([QL * PMOM, QH], F32)  # [32, 24] moment accumulator

    def prep_chunk(k):
        # packed int32 indices from DRAM; unpack to 3 int16 table indices.
        # idx is the raw per-core [per] array (no host padding): full chunks
        # slice 8192 ints; the ragged last chunk zero-fills its tail.
        vt = pool.tile([128, cc], I32, tag="vt")
        base = k * 128 * cc
        if per is None or k < chunks - 1:
            nc.sync.dma_start(
                out=vt[:],
                in_=idx_in[base : base + 128 * cc].rearrange("(p j) -> p j", j=cc),
            )
        else:
            rem = per - base
            p_full = rem // cc
            r_rem = rem - p_full * cc
            nc.vector.memset(vt[:], 0)
            nc.sync.dma_start(
                out=vt[0:p_full, :],
                in_=idx_in[base : base + p_full * cc].rearrange("(p j) -> p j", j=cc),
            )
            if r_rem:
                nc.scalar.dma_start(
                    out=vt[p_full : p_full + 1, 0:r_rem],
                    in_=idx_in[base + p_full * cc : base + rem].rearrange(
                        "(p j) -> p j", p=1
                    ),
                )
        fs = pool.tile([128, cc], I32, tag="fs")
        nc.vector.tensor_scalar(
            out=fs[:], in0=vt[:], scalar1=7, scalar2=9,
            op0=OP.bitwise_and, op1=OP.logical_shift_left,
        )
        idxs16 = idxs16_raw[k % 2].ap()
        i16v = idxs16.rearrange("p (j s) -> p j s", s=3)
        idx_copies = []
        at = pool.tile([128, cc], I32, tag="at")
        tt_ = pool.tile([128, cc], I32, tag="tt")
        for si, shift in enumerate((3, 12, 21)):
            nc.vector.tensor_scalar(
                out=at[:], in0=vt[:], scalar1=shift, scalar2=511,
                op0=OP.logical_shift_right, op1=OP.bitwise_and,
            )
            nc.vector.tensor_tensor(out=tt_[:], in0=fs[:], in1=at[:], op=OP.add)
            cp = nc.vector.tensor_copy(out=i16v[:, :, si], in_=tt_[:])
            if k - 2 in prev_gather:  # WAR: slot reuse (2-deep raw buffers)
                add_dep_helper(cp.ins, prev_gather[k - 2].ins, reason="idxs16 WAR")
            idx_copies.append(cp)

        mask = maskl if k == chunks - 1 else None

        # GPSIMD gather: per 16-partition group g the idx stream unwraps as
        # n = m*16 + w (w = source partition%16, m = 3j+s); every partition p
        # of the group gathers the full stream from ITS table row (coord p%3)
        gath = gath_raw[k % 2].ap()
        # j-minor slot order spreads last-chunk pad across partitions, so the
        # full column range must be gathered; pad slots (idx 0) gather finite
        # atom-0 coords and are mask-zeroed downstream.
        ncols = cc
        gth = nc.gpsimd.ap_gather(
            out_ap=gath.unsqueeze(2),
            in_ap=xyzt_sb.ap().unsqueeze(2),
            idxs_ap=idxs16,
            channels=128,
            num_elems=TBL,
            d=1,
            num_idxs=3 * 16 * ncols,
        )
        for tdma in table_dmas:
            add_dep_helper(gth.ins, tdma.ins, reason="gather reads table")
        for cp in idx_copies:
            add_dep_helper(gth.ins, cp.ins, reason="gather reads idxs")
        if k - 2 in prev_repack:  # WAW on gath slot (2-deep raw buffers)
            for rp in prev_repack[k - 2]:
                add_dep_helper(gth.ins, rp.ins, reason="gath WAR vs old repack")
        prev_gather[k] = gth
        return gath, gth, mask

    prepped = {0: prep_chunk(0)}
    for k in range(chunks):
        # issue next chunk's prep + gather BEFORE this chunk's math so the
        # Pool engine (bottleneck) is never starved by DVE trace order
        if k + 1 < chunks:
            prepped[k + 1] = prep_chunk(k + 1)
        gath, gth, mask = prepped.pop(k)

        # contiguous-block repack: math partition p' = 16g + w' takes stream
        # block n in [w'*3cc, (w'+1)*3cc) of its group from rep partition
        # 16g+c. Block = whole triplets since 3cc % 48 == 0. One contiguous
        # DMA per coordinate. In-block: n - w'*3cc = 48*jj + 16*s + w0, the
        # angle being (partition 16g+w0, col 4w'+jj).
        gc = []
        repacks = []
        # three engines: sync/scalar get their own Pool-sem waits; gpsimd
        # follows the gather in Pool program order. (A single engine would
        # leave repacks 2-3 wait-free and racing the gather across queues.)
        rp_engines = [nc.sync, nc.scalar, nc.sync]
        for c3 in range(3):
            gt = pool.tile([128, 3 * cc], F32, tag=f"gc{c3}")
            rp = rp_engines[c3].dma_start(out=gt[:], in_=gath[c3:128:16, :])
            add_dep_helper(rp.ins, gth.ins, reason="repack reads gather output")
            repacks.append(rp)
            gc.append(gt)
        prev_repack[k] = repacks

        if k == 0 and "dbg_g" in outs:
            nc.sync.dma_start(out=outs["dbg_g"][:], in_=gc[0][:])

        # per-(coord, slot) views [128, jj(4), w0(16)] -> 64 angles/partition
        na = cc  # angles per partition per chunk (4*16)
        def sv(ci, si):
            return gc[ci][:].rearrange("p (j s w) -> p j s w", s=3, w=16)[:, :, si, :]

        d11 = pool.tile([128, na], F32, tag="d11")
        d22 = pool.tile([128, na], F32, tag="d22")
        d12 = pool.tile([128, na], F32, tag="d12")
        d11v = d11[:].rearrange("p (j w) -> p j w", w=16)
        d22v = d22[:].rearrange("p (j w) -> p j w", w=16)
        d12v = d12[:].rearrange("p (j w) -> p j w", w=16)
        v1c = pool.tile([128, cc // 16, 16], F32, tag="v1c")
        v2c = pool.tile([128, cc // 16, 16], F32, tag="v2c")
        mm = pool.tile([128, cc // 16, 16], F32, tag="mm")
        for ci in range(3):
            nc.vector.tensor_tensor(out=v1c[:], in0=sv(ci, 0), in1=sv(ci, 1), op=OP.subtract)
            nc.vector.tensor_tensor(out=v2c[:], in0=sv(ci, 2), in1=sv(ci, 1), op=OP.subtract)
            if ci == 0:
                nc.vector.tensor_tensor(out=d11v, in0=v1c[:], in1=v1c[:], op=OP.mult)
                nc.vector.tensor_tensor(out=d22v, in0=v2c[:], in1=v2c[:], op=OP.mult)
                nc.vector.tensor_tensor(out=d12v, in0=v1c[:], in1=v2c[:], op=OP.mult)
            else:
                nc.vector.tensor_tensor(out=mm[:], in0=v1c[:], in1=v1c[:], op=OP.mult)
                nc.vector.tensor_tensor(out=d11v, in0=d11v, in1=mm[:], op=OP.add)
                nc.vector.tensor_tensor(out=mm[:], in0=v2c[:], in1=v2c[:], op=OP.mult)
                nc.vector.tensor_tensor(out=d22v, in0=d22v, in1=mm[:], op=OP.add)
                nc.vector.tensor_tensor(out=mm[:], in0=v1c[:], in1=v2c[:], op=OP.mult)
                nc.vector.tensor_tensor(out=d12v, in0=d12v, in1=mm[:], op=OP.add)

        nn_ = pool.tile([128, cc], F32, tag="nn")
        nc.vector.tensor_tensor(out=nn_[:], in0=d11[:], in1=d22[:], op=OP.mult)
        sq = pool.tile([128, cc], F32, tag="sq")
        # bias keeps padded slots (zero vectors) finite: 1/sqrt(tiny) != inf*0
        nc.scalar.activation(sq[:], nn_[:], AF.Sqrt, bias=coefs[:, 8:9])
        rs = pool.tile([128, cc], F32, tag="rs")
        nc.vector.reciprocal(rs[:], sq[:])
        u = pool.tile([128, cc], F32, tag="u")
        nc.vector.tensor_tensor(out=u[:], in0=d12[:], in1=rs[:], op=OP.mult)
        # clamp |u| <= 1
        au0 = pool.tile([128, cc], F32, tag="au0")
        nc.scalar.activation(au0[:], u[:], AF.Abs)
        au = pool.tile([128, cc], F32, tag="au")
        nc.vector.tensor_scalar(
            out=au[:], in0=au0[:], scalar1=1.0, scalar2=None, op0=OP.min
        )
        sg = pool.tile([128, cc], F32, tag="sg")
        nc.scalar.activation(sg[:], u[:], AF.Sign)

        # theta_abs = sqrt(1-|u|) * P(|u|) in degrees (A&S 4.4.46, 8 terms);
        # theta = 90 + sg*(theta_abs - 90)
        sqterm = pool.tile([128, cc], F32, tag="sqterm")
        nc.scalar.activation(sqterm[:], au[:], AF.Sqrt, bias=1.0, scale=-1.0)
        x2 = pool.tile([128, cc], F32, tag="x2")
        nc.scalar.activation(x2[:], au[:], AF.Square)
        x4 = pool.tile([128, cc], F32, tag="x4")
        nc.scalar.activation(x4[:], x2[:], AF.Square)

        def pair(i_odd, col_even, tag):
            p = pool.tile([128, cc], F32, tag=tag)
            nc.vector.scalar_tensor_tensor(
                out=p[:], in0=au[:], scalar=float(ACOS_COEF[i_odd] * DEG),
                in1=coefs[:, col_even : col_even + 1].to_broadcast([128, cc]),
                op0=OP.mult, op1=OP.add,
            )
            return p

        p01 = pair(1, 0, "p01")
        p23 = pair(3, 2, "p23")
        p45 = pair(5, 4, "p45")
        p67 = pair(7, 6, "p67")
        t1 = pool.tile([128, cc], F32, tag="es1")
        nc.vector.tensor_tensor(out=t1[:], in0=x2[:], in1=p23[:], op=OP.mult)
        nc.vector.tensor_tensor(out=t1[:], in0=t1[:], in1=p01[:], op=OP.add)
        t2 = pool.tile([128, cc], F32, tag="es2")
        nc.vector.tensor_tensor(out=t2[:], in0=x2[:], in1=p67[:], op=OP.mult)
        nc.vector.tensor_tensor(out=t2[:], in0=t2[:], in1=p45[:], op=OP.add)
        nc.vector.tensor_tensor(out=t2[:], in0=t2[:], in1=x4[:], op=OP.mult)
        nc.vector.tensor_tensor(out=t1[:], in0=t1[:], in1=t2[:], op=OP.add)
        thabs = pool.tile([128, cc], F32, tag="thabs")
        nc.vector.tensor_tensor(out=thabs[:], in0=sqterm[:], in1=t1[:], op=OP.mult)
        theta = pool.tile([128, cc], F32, tag="theta")
        nc.vector.tensor_scalar(
            out=theta[:], in0=thabs[:], scalar1=-90.0, scalar2=None, op0=OP.add
        )
        nc.vector.tensor_tensor(out=theta[:], in0=theta[:], in1=sg[:], op=OP.mult)
        nc.vector.tensor_scalar(
            out=theta[:], in0=theta[:], scalar1=90.0, scalar2=None, op0=OP.add
        )
        if k == 0 and "dbg_theta" in outs:
            nc.sync.dma_start(out=outs["dbg_theta"][:], in_=theta[:])

        # fine bin q = round(theta/H) (convert rounding handled by probe: trunc)
        qf_pre = pool.tile([128, cc], F32, tag="qfpre")
        nc.vector.tensor_scalar(
            out=qf_pre[:], in0=theta[:], scalar1=1.0 / H, scalar2=0.5,
            op0=OP.mult, op1=OP.add,
        )
        q_i = pool.tile([128, cc], I32, tag="qi")
        nc.vector.tensor_copy(out=q_i[:], in_=qf_pre[:])
        qf = pool.tile([128, cc], F32, tag="qf")
        nc.vector.tensor_copy(out=qf[:], in_=q_i[:])
        eps = pool.tile([128, cc], F32, tag="eps")
        nc.vector.scalar_tensor_tensor(
            out=eps[:], in0=qf[:], scalar=-H, in1=theta[:], op0=OP.mult, op1=OP.add
        )
        qh_i = pool.tile([128, cc], I32, tag="qhi")
        nc.vector.tensor_scalar(
            out=qh_i[:], in0=q_i[:], scalar1=int(math.log2(QL)), scalar2=None,
            op0=OP.arith_shift_right
        )
        ql_i = pool.tile([128, cc], I32, tag="qli")
        nc.vector.tensor_scalar(
            out=ql_i[:], in0=q_i[:], scalar1=QL - 1, scalar2=None, op0=OP.bitwise_and
        )

        # moment payload E = mask * (1, eps, eps^2, eps^3)
        ee = pool.tile([128, cc, PMOM], F32, tag="ee")
        if mask is None:
            nc.vector.memset(ee[:, :, 0], 1.0)
            nc.vector.tensor_copy(out=ee[:, :, 1], in_=eps[:])
        else:
            nc.vector.tensor_copy(out=ee[:, :, 0], in_=mask[:])
            nc.vector.tensor_tensor(
                out=ee[:, :, 1], in0=eps[:], in1=mask[:], op=OP.mult
            )
        nc.vector.tensor_tensor(
            out=ee[:, :, 2], in0=ee[:, :, 1], in1=eps[:], op=OP.mult
        )
        nc.vector.tensor_tensor(
            out=ee[:, :, 3], in0=ee[:, :, 2], in1=eps[:], op=OP.mult
        )

        # one-hots
        oh_ql = pool.tile([128, cc, QL], F32, tag="ohql")
        nc.vector.tensor_tensor(
            out=oh_ql[:],
            in0=ql_i[:].unsqueeze(2).to_broadcast([128, cc, QL]),
            in1=iota_ql[:].unsqueeze(1).to_broadcast([128, cc, QL]),
            op=OP.is_equal,
        )
        oh_qh = pool.tile([128, cc, QH], F32, tag="ohqh")
        nc.vector.tensor_tensor(
            out=oh_qh[:],
            in0=qh_i[:].unsqueeze(2).to_broadcast([128, cc, QH]),
            in1=iota_qh[:].unsqueeze(1).to_broadcast([128, cc, QH]),
            op=OP.is_equal,
        )
        # lhsT[m, (ql, pm)] = oh_ql[m, ql] * E[m, pm]
        lhs = pool.tile([128, cc, QL * PMOM], F32, tag="lhs")
        nc.vector.tensor_tensor(
            out=lhs[:],
            in0=oh_ql[:].unsqueeze(3).to_broadcast([128, cc, QL, PMOM]),
            in1=ee[:].unsqueeze(2).to_broadcast([128, cc, QL, PMOM]),
            op=OP.mult,
        )

        for j in range(cc):
            nc.tensor.matmul(
                out=psum_m[:],
                lhsT=lhs[:, j, :],
                rhs=oh_qh[:, j, :],
                start=(k == 0 and j == 0),
                stop=(k == chunks - 1 and j == cc - 1),
            )

    # ---- moments out ----
    m_sb = const_pool.tile([QL * PMOM, QH], F32)
    nc.vector.tensor_copy(out=m_sb[:], in_=psum_m[:])
    if host_reduce:
        # ship the tiny per-core moment block; reduce + reconstruct on host
        nc.sync.dma_start(out=outs["outm"][:], in_=m_sb[:])
        return
    m_local = dram_pool.tile([QL * PMOM, QH], F32)
    nc.sync.dma_start(out=m_local[:], in_=m_sb[:])
    m_red = dram_pool.tile([QL * PMOM, QH], F32)
    nc.gpsimd.collective_compute(
        "AllReduce",
        OP.add,
        replica_groups=[list(range(N_CORES))],
        ins=[m_local[:].opt()],
        outs=[m_red[:].opt()],
    )
    # reload flat: element kk = p*QH + n ; rhs chunks [128, 6]
    m_rhs = const_pool.tile([128, 6], F32)
    nc.sync.dma_start(
        out=m_rhs[:], in_=m_red[:].rearrange("p n -> (p n)").rearrange("(c p) -> p c", p=128)
    )

    # ---- final contraction count[b] = sum_k M[k] * A[k, b] ----
    psum_ca = psum_pool.tile([128, 1], F32)
    psum_cb = psum_pool.tile([NBINS - 128, 1], F32)
    for cquad in range(6):
        nc.tensor.matmul(
            out=psum_ca[:], lhsT=a_sb[:, cquad, 0:128], rhs=m_rhs[:, cquad : cquad + 1],
            start=(cquad == 0), stop=(cquad == 5),
        )
    for cquad in range(6):
        nc.tensor.matmul(
            out=psum_cb[:], lhsT=a_sb[:, cquad, 128:NBINS], rhs=m_rhs[:, cquad : cquad + 1],
            start=(cquad == 0), stop=(cquad == 5),
        )
    cnt = const_pool.tile([128, 2], F32)
    nc.vector.memset(cnt[:], 0.0)
    nc.vector.tensor_copy(out=cnt[:, 0:1], in_=psum_ca[:])
    nc.vector.tensor_copy(out=cnt[0 : NBINS - 128, 1:2], in_=psum_cb[:])

    # total + normalize
    psum_t = psum_pool.tile([1, 2], F32)
    nc.tensor.matmul(out=psum_t[:], lhsT=ones_col[:], rhs=cnt[:], start=True, stop=True)
    tt = const_pool.tile([1, 2], F32)
    nc.vector.tensor_copy(out=tt[:], in_=psum_t[:])
    tot = const_pool.tile([1, 1], F32)
    nc.vector.tensor_tensor(out=tot[:], in0=tt[:, 0:1], in1=tt[:, 1:2], op=OP.add)
    rtot = const_pool.tile([1, 1], F32)
    nc.vector.reciprocal(rtot[:], tot[:])
    psum_r = psum_pool.tile([128, 1], F32)
    nc.tensor.matmul(out=psum_r[:], lhsT=ones_row[:], rhs=rtot[:], start=True, stop=True)
    outn = const_pool.tile([128, 2], F32)
    nc.vector.tensor_tensor(
        out=outn[:], in0=cnt[:], in1=psum_r[:].to_broadcast([128, 2]), op=OP.mult
    )
    nc.sync.dma_start(out=out[0:128], in_=outn[:, 0])
    nc.sync.dma_start(out=out[128:NBINS], in_=outn[0 : NBINS - 128, 1])


# ---------------- host side ----------------

def prep_global_inputs(xyz: np.ndarray, angle_list: np.ndarray):
    """Pack host-side directly into the core-concatenated global arrays."""
    flat = np.asarray(xyz, dtype=np.float32).reshape(-1, 3)  # [4096, 3]
    xyz3 = np.ascontiguousarray(flat.T)  # [3, 4096]
    al = np.asarray(angle_list).astype(np.int32, copy=False)
    per = angle_list.shape[0] // N_CORES
    assert per == PER_CORE and per <= SLOTS
    # one int32 per angle: f | a1<<3 | c<<12 | a2<<21. The concatenated
    # global array IS v_all (j-minor slot order s' = (k*128 + p)*C + j per
    # core); the device zero-fills the ragged last chunk, so no host padding
    # or copies are needed.
    v_all = (
        al[:, 0] | (al[:, 1] << 3) | (al[:, 2] << 12) | (al[:, 3] << 21)
    ).astype(np.int32)
    return {"idx": v_all, "xyz3": xyz3}


_PROG_CACHE = {}


def build_program(chunks=CHUNKS, cols=C):
    key = (chunks, cols, _HOST_REDUCE)
    if key in _PROG_CACHE:
        return _PROG_CACHE[key]
    nc = bacc.Bacc("TRN2", target_bir_lowering=False, num_devices=N_CORES)
    idx_len = PER_CORE if chunks == CHUNKS else chunks * 128 * cols
    ins = {
        "idx": nc.dram_tensor("idx", [idx_len], I32, kind="ExternalInput").ap(),
        "xyz3": nc.dram_tensor("xyz3", [3, TBL], F32, kind="ExternalInput").ap(),
    }
    if _HOST_REDUCE:
        outs = {
            "outm": nc.dram_tensor(
                "outm", [QL * PMOM, QH], F32, kind="ExternalOutput"
            ).ap()
        }
    else:
        outs = {"out": nc.dram_tensor("out", [NBINS], F32, kind="ExternalOutput").ap()}
    # raw ap_gather buffers: must be allocated BEFORE TileContext so the tile
    # pools (which claim the free SBUF region at entry) don't overlap them.
    xyzt_sb = nc.alloc_sbuf_tensor("xyzt_sb", [128, TBL], F32)
    idxs16_raw = [
        nc.alloc_sbuf_tensor(f"idxs16r{i}", [128, 3 * cols], I16)
        for i in range(2)
    ]
    gath_raw = [
        nc.alloc_sbuf_tensor(f"gathr{i}", [128, 3 * 16 * cols], F32)
        for i in range(2)
    ]
    raw = (xyzt_sb, idxs16_raw, gath_raw)
    with tile.TileContext(nc) as tc:
        adf_kernel(tc, outs, ins, raw, per=PER_CORE if chunks == CHUNKS else None)
    nc.compile()
    _PROG_CACHE[key] = nc
    return nc


# ---------------- cached PJRT runner ----------------
# run_bass_kernel_spmd rebuilds its jax.jit closure on every call (full
# retrace + lowering each time). Build the jitted sharded callable ONCE and
# reuse it; each call still ships the packed inputs and runs the device.

_RUNNER = None


def _get_runner():
    global _RUNNER
    if _RUNNER is not None:
        return _RUNNER
    import jax
    from jax.sharding import Mesh, PartitionSpec
    from jax.experimental.shard_map import shard_map
    from concourse.bass2jax import (
        _bass_exec_p, install_neuronx_cc_hook, partition_id_tensor,
    )

    nc = build_program()
    install_neuronx_cc_hook()

    partition_name = nc.partition_id_tensor.name if nc.partition_id_tensor else None
    in_names, out_names, out_avals = [], [], []
    for alloc in nc.m.functions[0].allocations:
        if not isinstance(alloc, mybir.MemoryLocationSet):
            continue
        name = alloc.memorylocations[0].name
        if alloc.kind == "ExternalInput":
            if name != partition_name:
                in_names.append(name)
        elif alloc.kind == "ExternalOutput":
            shape = tuple(alloc.tensor_shape)
            dtype = mybir.dt.np(alloc.dtype)
            out_names.append(name)
            out_avals.append(jax.core.ShapedArray(shape, dtype))
    assert nc.dbg_addr is None, "debug build unsupported in cached PJRT runner"
    # the NKI lowering only consumes ExternalInput-named operands and the
    # alias map is empty, so no donated zero-output operands are needed; the
    # kernel writes every element of 'out'.
    in_names_full = in_names + (
        [partition_name] if partition_name is not None else []
    )

    def _body(*args):
        operands = list(args)
        if partition_name is not None:
            operands.append(partition_id_tensor())
        outs_ = _bass_exec_p.bind(
            *operands,
            out_avals=tuple(out_avals),
            in_names=tuple(in_names_full),
            out_names=tuple(out_names),
            lowering_input_output_aliases=(),
            sim_require_finite=True,
            sim_require_nnan=True,
            nc=nc,
        )
        return tuple(outs_)

    devices = jax.devices()[:N_CORES]
    assert len(devices) == N_CORES
    mesh = Mesh(np.asarray(devices), ("core",))
    # xyz3 is identical on every core: ship one copy, replicated in_spec
    in_specs = tuple(
        PartitionSpec() if n == "xyz3" else PartitionSpec("core") for n in in_names
    )
    sharded = jax.jit(
        shard_map(
            _body, mesh=mesh,
            in_specs=in_specs,
            out_specs=(PartitionSpec("core"),) * len(out_names),
            check_rep=False,
        ),
        keep_unused=True,
    )

    out_pos = {name: i for i, name in enumerate(out_names)}

    def run(global_map):
        concat_in = [global_map[name] for name in in_names]
        out_arrs = sharded(*concat_in)
        if "outm" in out_pos:
            # per-core moment blocks: fetch all 8 shards, reduce on host
            return np.asarray(out_arrs[out_pos["outm"]])
        # fetch only core 0's shard of 'out' (all cores produce identical output)
        shard = out_arrs[out_pos["out"]].addressable_shards[0].data
        return np.asarray(shard)

    _RUNNER = run
    return run


_AMAT = None


def kernel(**inputs) -> np.ndarray:
    global _AMAT
    xyz = np.asarray(inputs["xyz"], dtype=np.float32)
    angle_list = np.asarray(inputs["angle_list"])
    run = _get_runner()  # one-time program build + jit construction
    if _HOST_REDUCE and _AMAT is None:
        _AMAT = build_amat().astype(np.float64)
    # timed region matches the baseline definition: device dispatch + transfer
    # + exec + result fetch (host packing excluded, as in the original)
    gm = prep_global_inputs(xyz, angle_list)
    t0 = time.time()
    out = run(gm)
    if _HOST_REDUCE:
        m = out.reshape(N_CORES, QL * PMOM * QH).sum(axis=0, dtype=np.float64)
        count = m @ _AMAT
        out = (count / count.sum()).astype(np.float32)
    kernel._last_run_s = time.time() - t0
    kernel._last_results = None
    return np.asarray(out, dtype=np.float32)


if __name__ == "__main__":
    # smoke: build only
    build_program()
    print("program built ok")


# revision 36
# speedup vs baseline: 1.0676x; 1.0351x over previous
"""Trainium2 Bass kernel for nn_DifferentiableADF (angular distribution function).

Computes: for M=500k angle triplets over xyz[8,512,3], the Gaussian-smeared
180-bin histogram of bond angles, normalized to sum 1.

Strategy (8 cores, data-parallel over angles):
  - per-call host->device traffic is minimized (it dominates wall time under
    the axon-proxied PJRT transport: ~70ms fixed RPC legs + ~12ms/MB): each
    angle ships as ONE packed int32 (f | a1<<3 | c<<12 | a2<<21, j-minor slot
    order), and the concatenated global array is exactly the packed [500000]
    vector — no host padding or copies; the device zero-fills the ragged last
    chunk. Plus one replicated [3, 4096] raw xyz table (48KB). Everything
    else (index unpack to int16, table replication to the 128-partition
    coordinate-split layout, Gaussian/Hermite derivative matrix, acos
    coefficients, last-chunk validity mask) is computed on device each run
    for ~0 cost. Total shipped: 2.0MB, the entropy of the angle data.
  - per chunk: DVE unpacks the int32 into 3 int16 table indices; GPSIMD
    ap_gather fetches the 9 coords per angle; a contiguous-block DMA repack
    aligns the stream to compute partitions. Bond vectors + dots on DVE,
    arccos via A&S 4.4.46 polynomial, fast-Gauss-transform moment
    accumulation: theta -> nearest fine bin q (the 180-bin output grid
    itself), moments (1, eps, eps^2, eps^3) scattered into bins via a
    digit-split one-hot matmul on the PE (PSUM accumulates across chunks).
  - AllReduce of the [32,24] moment block, then a tiny matmul against the
    on-device-generated Hermite-derivative matrix reconstructs the exact
    smeared histogram; normalized on device. All cores produce identical
    output; only core 0's shard is fetched.
  - the jitted PJRT callable is built once per process and reused (the
    library helper re-traces/re-lowers per call); warm calls pay only input
    packing (~3ms) + transfer + dispatch + device exec (~0.1ms).

NOTE: f32->i32 tensor_copy on DVE rounds to nearest-even (NOT truncate);
on-device integer digit extraction uses tie-free offsets to get exact floors.
"""

import math
import os
import sys
import time
from contextlib import ExitStack

import numpy as np

sys.path.insert(0, "/opt/trn_rl_repo")

import concourse.bass as bass  # noqa: E402
import concourse.tile as tile  # noqa: E402
from concourse.tile import add_dep_helper  # noqa: E402
from concourse import bacc, mybir  # noqa: E402
from concourse._compat import with_exitstack  # noqa: E402

F32 = mybir.dt.float32
I32 = mybir.dt.int32
I16 = mybir.dt.int16
AF = mybir.ActivationFunctionType
OP = mybir.AluOpType

# ---------------- problem constants ----------------
N_FRAMES = 8
N_ATOMS = 512
N_ANGLES = 500_000
NBINS = 180
H = 180.0 / 179.0  # bin spacing == fine-grid spacing
N_CORES = 8
PER_CORE = N_ANGLES // N_CORES  # 62500
TBL = N_FRAMES * N_ATOMS  # 4096

QL = 8   # low digit of fine-bin index
QH = 24  # high digit (8*24 = 192 >= 180 bins; q in [0,191] all valid rows)
PMOM = 4  # moments kept: eps^0..eps^3
KFLAT = QL * PMOM * QH  # 768 = 6*128
DEG = 180.0 / math.pi

# layout: angle slot s = ((p*CHUNKS + k)*C + j)  p: partition, k: chunk, j: col
CHUNKS = 8
C = 64  # must be multiple of 16 (contiguous-block repack needs 3C % 48 == 0)
SLOTS = 128 * CHUNKS * C  # 65536 >= 62500

# Abramowitz & Stegun 4.4.46: arccos(x) = sqrt(1-x) * sum a_k x^k, x in [0,1]
ACOS_COEF = [
    1.5707963050, -0.2145988016, 0.0889789874, -0.0501743046,
    0.0308918810, -0.0170881256, 0.0066700901, -0.0012624911,
]

_SIM_INIT_RAW = False  # set True (before build) for CoreSim runs only
# True: device returns per-core [32,24] moment blocks, host does the
# reduce + Hermite reconstruction + normalize (no on-device collective).
_HOST_REDUCE = False


def build_amat() -> np.ndarray:
    """A[(ql*PMOM+pm)*QH+qh, b] = g^(pm)(c_q - o_b)/pm!  with g = exp(-x^2/2)."""
    q = np.arange(QL * QH, dtype=np.float64)
    b = np.arange(NBINS, dtype=np.float64)
    d = q[:, None] * H - b[None, :] * H  # [192, 180]
    g0 = np.exp(-0.5 * d * d)
    derivs = [g0, -d * g0, (d * d - 1.0) / 2.0 * g0, (3.0 * d - d**3) / 6.0 * g0]
    a = np.zeros((KFLAT, NBINS), dtype=np.float64)
    for qi in range(QL * QH):
        ql, qh = qi % QL, qi // QL
        for pm in range(PMOM):
            a[(ql * PMOM + pm) * QH + qh, :] = derivs[pm][qi, :]
    return a.astype(np.float32)


@with_exitstack
def adf_kernel(ctx: ExitStack, tc: tile.TileContext, outs, ins, raw, per=None):
    nc = tc.nc
    xyzt_sb, idxs16_raw, gath_raw = raw
    idx_in = ins["idx"]      # [per] int32 packed f|a1<<3|c<<12|a2<<21, j-minor
    xyz3 = ins["xyz3"]       # [3, TBL] f32 raw coordinate-split table
    host_reduce = "outm" in outs
    out = None if host_reduce else outs["out"]  # [180] f32

    chunks, cc = CHUNKS, C

    const_pool = ctx.enter_context(tc.tile_pool(name="const", bufs=1))
    gen_pool = ctx.enter_context(tc.tile_pool(name="gen", bufs=1))
    pool = ctx.enter_context(tc.tile_pool(name="work", bufs=3))
    psum_pool = ctx.enter_context(tc.tile_pool(name="psum", bufs=1, space="PSUM"))
    dram_pool = ctx.enter_context(tc.tile_pool(name="dram", bufs=1, space="DRAM"))

    if _SIM_INIT_RAW:  # CoreSim rejects reads of uninit SBUF; HW tolerates
        for gb in gath_raw:
            nc.vector.memset(gb.ap(), 0.0)

    # ---- replicate the [3, TBL] raw table to partition p = coord p%3 ----
    table_dmas = []
    rep_engines = [nc.sync, nc.scalar]
    for r in range(42):
        td = rep_engines[r % 2].dma_start(
            out=xyzt_sb.ap()[3 * r : 3 * r + 3], in_=xyz3[:]
        )
        table_dmas.append(td)
    td = nc.sync.dma_start(out=xyzt_sb.ap()[126:128], in_=xyz3[0:2])
    table_dmas.append(td)

    # ---- constants ----
    iota_ql = const_pool.tile([128, QL], I32)
    nc.gpsimd.iota(iota_ql[:], pattern=[[1, QL]], base=0, channel_multiplier=0)
    iota_qh = const_pool.tile([128, QH], I32)
    nc.gpsimd.iota(iota_qh[:], pattern=[[1, QH]], base=0, channel_multiplier=0)
    ones_col = const_pool.tile([128, 1], F32)
    nc.vector.memset(ones_col[:], 1.0)
    ones_row = const_pool.tile([1, 128], F32)
    nc.vector.memset(ones_row[:], 1.0)

    # acos coefficients (DEG-scaled, even terms) + tiny-bias constant
    coefs = const_pool.tile([128, 12], F32)
    for col in (0, 2, 4, 6):
        nc.vector.memset(coefs[:, col : col + 1], float(ACOS_COEF[col] * DEG))
    nc.vector.memset(coefs[:, 8:9], 1e-30)

    # ---- last-chunk validity mask, generated on device ----
    # j-minor slot order s' = (k*128 + p)*C + j, valid iff s' < per; math slot
    # (p'=16g+w', col 16jj+w0) has p=16g+w0, j=4w'+jj, so
    # s' - base = p*C + j = (g<<10) + (w0<<6) + (w'<<2) + jj
    maskl = const_pool.tile([128, cc], F32)
    if per is not None:
        m_ci = gen_pool.tile([128, cc], I32)
        nc.gpsimd.iota(m_ci[:], pattern=[[1, cc]], base=0, channel_multiplier=0)
        m_pi = gen_pool.tile([128, 1], I32)
        nc.gpsimd.iota(m_pi[:], pattern=[[1, 1]], base=0, channel_multiplier=1)
        m_a1 = gen_pool.tile([128, 1], I32)
        nc.vector.tensor_scalar(
            out=m_a1[:], in0=m_pi[:], scalar1=15, scalar2=2,
            op0=OP.bitwise_and, op1=OP.logical_shift_left,
        )
        m_a2 = gen_pool.tile([128, 1], I32)
        nc.vector.tensor_scalar(
            out=m_a2[:], in0=m_pi[:], scalar1=4, scalar2=10,
            op0=OP.logical_shift_right, op1=OP.logical_shift_left,
        )
        m_ap = gen_pool.tile([128, 1], I32)
        nc.vector.tensor_tensor(out=m_ap[:], in0=m_a1[:], in1=m_a2[:], op=OP.add)
        m_b1 = gen_pool.tile([128, cc], I32)
        nc.vector.tensor_scalar(
            out=m_b1[:], in0=m_ci[:], scalar1=15, scalar2=6,
            op0=OP.bitwise_and, op1=OP.logical_shift_left,
        )
        m_b2 = gen_pool.tile([128, cc], I32)
        nc.vector.tensor_scalar(
            out=m_b2[:], in0=m_ci[:], scalar1=4, scalar2=None,
            op0=OP.logical_shift_right,
        )
        m_val = gen_pool.tile([128, cc], I32)
        nc.vector.tensor_tensor(out=m_val[:], in0=m_b1[:], in1=m_b2[:], op=OP.add)
        nc.vector.tensor_tensor(
            out=m_val[:], in0=m_val[:], in1=m_ap[:].to_broadcast([128, cc]), op=OP.add
        )
        base = (chunks - 1) * cc * 128
        nc.vector.tensor_scalar(
            out=maskl[:], in0=m_val[:], scalar1=base, scalar2=per,
            op0=OP.add, op1=OP.is_lt,
        )
    else:
        nc.vector.memset(maskl[:], 1.0)

    # ---- generate A on device: a_sb[p, c, b] = A[c*128+p, b] ----
    # flat row r = c*128+p = (ql*PMOM+pm)*QH + qh
    a_sb = const_pool.tile([128, 6, NBINS], F32)
    r_i = gen_pool.tile([128, 6], I32)
    nc.gpsimd.iota(r_i[:], pattern=[[128, 6]], base=0, channel_multiplier=1)
    r_f = gen_pool.tile([128, 6], F32)
    nc.vector.tensor_copy(out=r_f[:], in_=r_i[:])
    # NOTE: f32->i32 tensor_copy rounds to nearest-even. The offsets below are
    # tie-free for the value grids (j/24 resp. j/4), so rint(x - off) == floor(x).
    tmp6 = gen_pool.tile([128, 6], F32)
    nc.vector.tensor_scalar(
        out=tmp6[:], in0=r_f[:], scalar1=float(np.float32(1.0 / 24.0)),
        scalar2=0.47916667, op0=OP.mult, op1=OP.subtract,
    )
    t24i = gen_pool.tile([128, 6], I32)
    nc.vector.tensor_copy(out=t24i[:], in_=tmp6[:])  # rne -> exact floor(r/24)
    t24f = gen_pool.tile([128, 6], F32)
    nc.vector.tensor_copy(out=t24f[:], in_=t24i[:])
    qh_f = gen_pool.tile([128, 6], F32)
    nc.vector.scalar_tensor_tensor(
        out=qh_f[:], in0=t24f[:], scalar=-24.0, in1=r_f[:], op0=OP.mult, op1=OP.add
    )
    nc.vector.tensor_scalar(
        out=tmp6[:], in0=t24f[:], scalar1=0.25, scalar2=0.375,
        op0=OP.mult, op1=OP.subtract,
    )
    qli = gen_pool.tile([128, 6], I32)
    nc.vector.tensor_copy(out=qli[:], in_=tmp6[:])  # rne -> exact floor(t24/4)
    ql_f = gen_pool.tile([128, 6], F32)
    nc.vector.tensor_copy(out=ql_f[:], in_=qli[:])
    pm_f = gen_pool.tile([128, 6], F32)
    nc.vector.scalar_tensor_tensor(
        out=pm_f[:], in0=ql_f[:], scalar=-4.0, in1=t24f[:], op0=OP.mult, op1=OP.add
    )
    qv = gen_pool.tile([128, 6], F32)
    nc.vector.scalar_tensor_tensor(
        out=qv[:], in0=qh_f[:], scalar=float(QL), in1=ql_f[:], op0=OP.mult, op1=OP.add
    )
    # pm one-hot masks
    pmmask = []
    for k in range(PMOM):
        mc = gen_pool.tile([128, 1], F32, tag=f"amc{k}")
        nc.vector.memset(mc[:], float(k))
        mk = gen_pool.tile([128, 6], F32, tag=f"amk{k}")
        nc.vector.tensor_tensor(
            out=mk[:], in0=pm_f[:], in1=mc[:].to_broadcast([128, 6]), op=OP.is_equal
        )
        pmmask.append(mk)
    # b column index
    b_i = gen_pool.tile([128, NBINS], I32)
    nc.gpsimd.iota(b_i[:], pattern=[[1, NBINS]], base=0, channel_multiplier=0)
    b_f = gen_pool.tile([128, NBINS], F32)
    nc.vector.tensor_copy(out=b_f[:], in_=b_i[:])
    # d = (q - b) * H   [128, 6, NBINS]
    d_t = gen_pool.tile([128, 6, NBINS], F32)
    nc.vector.tensor_tensor(
        out=d_t[:],
        in0=qv[:].unsqueeze(2).to_broadcast([128, 6, NBINS]),
        in1=b_f[:].unsqueeze(1).to_broadcast([128, 6, NBINS]),
        op=OP.subtract,
    )
    nc.vector.tensor_scalar(
        out=d_t[:], in0=d_t[:], scalar1=float(np.float32(H)), scalar2=None, op0=OP.mult
    )
    d2_t = gen_pool.tile([128, 6, NBINS], F32)
    nc.vector.tensor_tensor(out=d2_t[:], in0=d_t[:], in1=d_t[:], op=OP.mult)
    g0_t = gen_pool.tile([128, 6, NBINS], F32)
    nc.scalar.activation(g0_t[:], d2_t[:], AF.Exp, scale=-0.5)
    t2_t = gen_pool.tile([128, 6, NBINS], F32)
    nc.vector.tensor_scalar(
        out=t2_t[:], in0=d2_t[:], scalar1=-1.0, scalar2=0.5, op0=OP.add, op1=OP.mult
    )
    u3_t = gen_pool.tile([128, 6, NBINS], F32)
    nc.vector.tensor_scalar(
        out=u3_t[:], in0=d2_t[:], scalar1=-1.0, scalar2=3.0, op0=OP.mult, op1=OP.add
    )
    t3_t = gen_pool.tile([128, 6, NBINS], F32)
    nc.vector.scalar_tensor_tensor(
        out=t3_t[:], in0=d_t[:], scalar=float(np.float32(1.0 / 6.0)), in1=u3_t[:],
        op0=OP.mult, op1=OP.mult,
    )
    acc_t = gen_pool.tile([128, 6, NBINS], F32)
    nc.vector.scalar_tensor_tensor(
        out=acc_t[:], in0=d_t[:], scalar=-1.0,
        in1=pmmask[1][:].unsqueeze(2).to_broadcast([128, 6, NBINS]),
        op0=OP.mult, op1=OP.mult,
    )
    nc.vector.tensor_tensor(
        out=acc_t[:], in0=acc_t[:],
        in1=pmmask[0][:].unsqueeze(2).to_broadcast([128, 6, NBINS]), op=OP.add
    )
    tmp_t = gen_pool.tile([128, 6, NBINS], F32)
    nc.vector.tensor_tensor(
        out=tmp_t[:], in0=t2_t[:],
        in1=pmmask[2][:].unsqueeze(2).to_broadcast([128, 6, NBINS]), op=OP.mult
    )
    nc.vector.tensor_tensor(out=acc_t[:], in0=acc_t[:], in1=tmp_t[:], op=OP.add)
    nc.vector.tensor_tensor(
        out=tmp_t[:], in0=t3_t[:],
        in1=pmmask[3][:].unsqueeze(2).to_broadcast([128, 6, NBINS]), op=OP.mult
    )
    nc.vector.tensor_tensor(out=acc_t[:], in0=acc_t[:], in1=tmp_t[:], op=OP.add)
    nc.vector.tensor_tensor(out=a_sb[:], in0=acc_t[:], in1=g0_t[:], op=OP.mult)
    if "dbg_amat" in outs:
        nc.sync.dma_start(out=outs["dbg_amat"][:], in_=a_sb[:])

    prev_gather = {}  # chunk -> gather inst (ap_gather APs invisible to Tile)
    prev_repack = {}  # chunk -> [repack insts]

    psum_m = psum_pool.tile([QL * PMOM, QH], F32)  # [32, 24] moment accumulator

    def prep_chunk(k):
        # packed int32 indices from DRAM; unpack to 3 int16 table indices.
        # idx is the raw per-core [per] array (no host padding): full chunks
        # slice 8192 ints; the ragged last chunk zero-fills its tail.
        vt = pool.tile([128, cc], I32, tag="vt")
        base = k * 128 * cc
        if per is None or k < chunks - 1:
            nc.sync.dma_start(
                out=vt[:],
                in_=idx_in[base : base + 128 * cc].rearrange("(p j) -> p j", j=cc),
            )
        else:
            rem = per - base
            p_full = rem // cc
            r_rem = rem - p_full * cc
            nc.vector.memset(vt[:], 0)
            nc.sync.dma_start(
                out=vt[0:p_full, :],
                in_=idx_in[base : base + p_full * cc].rearrange("(p j) -> p j", j=cc),
            )
            if r_rem:
                nc.scalar.dma_start(
                    out=vt[p_full : p_full + 1, 0:r_rem],
                    in_=idx_in[base + p_full * cc : base + rem].rearrange(
                        "(p j) -> p j", p=1
                    ),
                )
        fs = pool.tile([128, cc], I32, tag="fs")
        nc.vector.tensor_scalar(
            out=fs[:], in0=vt[:], scalar1=7, scalar2=9,
            op0=OP.bitwise_and, op1=OP.logical_shift_left,
        )
        idxs16 = idxs16_raw[k % 2].ap()
        i16v = idxs16.rearrange("p (j s) -> p j s", s=3)
        idx_copies = []
        at = pool.tile([128, cc], I32, tag="at")
        tt_ = pool.tile([128, cc], I32, tag="tt")
        for si, shift in enumerate((3, 12, 21)):
            nc.vector.tensor_scalar(
                out=at[:], in0=vt[:], scalar1=shift, scalar2=511,
                op0=OP.logical_shift_right, op1=OP.bitwise_and,
            )
            nc.vector.tensor_tensor(out=tt_[:], in0=fs[:], in1=at[:], op=OP.add)
            cp = nc.vector.tensor_copy(out=i16v[:, :, si], in_=tt_[:])
            if k - 2 in prev_gather:  # WAR: slot reuse (2-deep raw buffers)
                add_dep_helper(cp.ins, prev_gather[k - 2].ins, reason="idxs16 WAR")
            idx_copies.append(cp)

        mask = maskl if k == chunks - 1 else None

        # GPSIMD gather: per 16-partition group g the idx stream unwraps as
        # n = m*16 + w (w = source partition%16, m = 3j+s); every partition p
        # of the group gathers the full stream from ITS table row (coord p%3)
        gath = gath_raw[k % 2].ap()
        # j-minor slot order spreads last-chunk pad across partitions, so the
        # full column range must be gathered; pad slots (idx 0) gather finite
        # atom-0 coords and are mask-zeroed downstream.
        ncols = cc
        gth = nc.gpsimd.ap_gather(
            out_ap=gath.unsqueeze(2),
            in_ap=xyzt_sb.ap().unsqueeze(2),
            idxs_ap=idxs16,
            channels=128,
            num_elems=TBL,
            d=1,
            num_idxs=3 * 16 * ncols,
        )
        for tdma in table_dmas:
            add_dep_helper(gth.ins, tdma.ins, reason="gather reads table")
        for cp in idx_copies:
            add_dep_helper(gth.ins, cp.ins, reason="gather reads idxs")
        if k - 2 in prev_repack:  # WAW on gath slot (2-deep raw buffers)
            for rp in prev_repack[k - 2]:
                add_dep_helper(gth.ins, rp.ins, reason="gath WAR vs old repack")
        prev_gather[k] = gth
        return gath, gth, mask

    prepped = {0: prep_chunk(0)}
    for k in range(chunks):
        # issue next chunk's prep + gather BEFORE this chunk's math so the
        # Pool engine (bottleneck) is never starved by DVE trace order
        if k + 1 < chunks:
            prepped[k + 1] = prep_chunk(k + 1)
        gath, gth, mask = prepped.pop(k)

        # contiguous-block repack: math partition p' = 16g + w' takes stream
        # block n in [w'*3cc, (w'+1)*3cc) of its group from rep partition
        # 16g+c. Block = whole triplets since 3cc % 48 == 0. One contiguous
        # DMA per coordinate. In-block: n - w'*3cc = 48*jj + 16*s + w0, the
        # angle being (partition 16g+w0, col 4w'+jj).
        gc = []
        repacks = []
        # three engines: sync/scalar get their own Pool-sem waits; gpsimd
        # follows the gather in Pool program order. (A single engine would
        # leave repacks 2-3 wait-free and racing the gather across queues.)
        rp_engines = [nc.sync, nc.scalar, nc.sync]
        for c3 in range(3):
            gt = pool.tile([128, 3 * cc], F32, tag=f"gc{c3}")
            rp = rp_engines[c3].dma_start(out=gt[:], in_=gath[c3:128:16, :])
            add_dep_helper(rp.ins, gth.ins, reason="repack reads gather output")
            repacks.append(rp)
            gc.append(gt)
        prev_repack[k] = repacks

        if k == 0 and "dbg_g" in outs:
            nc.sync.dma_start(out=outs["dbg_g"][:], in_=gc[0][:])

        # per-(coord, slot) views [128, jj(4), w0(16)] -> 64 angles/partition
        na = cc  # angles per partition per chunk (4*16)
        def sv(ci, si):
            return gc[ci][:].rearrange("p (j s w) -> p j s w", s=3, w=16)[:, :, si, :]

        d11 = pool.tile([128, na], F32, tag="d11")
        d22 = pool.tile([128, na], F32, tag="d22")
        d12 = pool.tile([128, na], F32, tag="d12")
        d11v = d11[:].rearrange("p (j w) -> p j w", w=16)
        d22v = d22[:].rearrange("p (j w) -> p j w", w=16)
        d12v = d12[:].rearrange("p (j w) -> p j w", w=16)
        v1c = pool.tile([128, cc // 16, 16], F32, tag="v1c")
        v2c = pool.tile([128, cc // 16, 16], F32, tag="v2c")
        mm = pool.tile([128, cc // 16, 16], F32, tag="mm")
        for ci in range(3):
            nc.vector.tensor_tensor(out=v1c[:], in0=sv(ci, 0), in1=sv(ci, 1), op=OP.subtract)
            nc.vector.tensor_tensor(out=v2c[:], in0=sv(ci, 2), in1=sv(ci, 1), op=OP.subtract)
            if ci == 0:
                nc.vector.tensor_tensor(out=d11v, in0=v1c[:], in1=v1c[:], op=OP.mult)
                nc.vector.tensor_tensor(out=d22v, in0=v2c[:], in1=v2c[:], op=OP.mult)
                nc.vector.tensor_tensor(out=d12v, in0=v1c[:], in1=v2c[:], op=OP.mult)
            else:
                nc.vector.tensor_tensor(out=mm[:], in0=v1c[:], in1=v1c[:], op=OP.mult)
                nc.vector.tensor_tensor(out=d11v, in0=d11v, in1=mm[:], op=OP.add)
                nc.vector.tensor_tensor(out=mm[:], in0=v2c[:], in1=v2c[:], op=OP.mult)
                nc.vector.tensor_tensor(out=d22v, in0=d22v, in1=mm[:], op=OP.add)
                nc.vector.tensor_tensor(out=mm[:], in0=v1c[:], in1=v2c[:], op=OP.mult)
                nc.vector.tensor_tensor(out=d12v, in0=d12v, in1=mm[:], op=OP.add)

        nn_ = pool.tile([128, cc], F32, tag="nn")
        nc.vector.tensor_tensor(out=nn_[:], in0=d11[:], in1=d22[:], op=OP.mult)
        sq = pool.tile([128, cc], F32, tag="sq")
        # bias keeps padded slots (zero vectors) finite: 1/sqrt(tiny) != inf*0
        nc.scalar.activation(sq[:], nn_[:], AF.Sqrt, bias=coefs[:, 8:9])
        rs = pool.tile([128, cc], F32, tag="rs")
        nc.vector.reciprocal(rs[:], sq[:])
        u = pool.tile([128, cc], F32, tag="u")
        nc.vector.tensor_tensor(out=u[:], in0=d12[:], in1=rs[:], op=OP.mult)
        # clamp |u| <= 1
        au0 = pool.tile([128, cc], F32, tag="au0")
        nc.scalar.activation(au0[:], u[:], AF.Abs)
        au = pool.tile([128, cc], F32, tag="au")
        nc.vector.tensor_scalar(
            out=au[:], in0=au0[:], scalar1=1.0, scalar2=None, op0=OP.min
        )
        sg = pool.tile([128, cc], F32, tag="sg")
        nc.scalar.activation(sg[:], u[:], AF.Sign)

        # theta_abs = sqrt(1-|u|) * P(|u|) in degrees (A&S 4.4.46, 8 terms);
        # theta = 90 + sg*(theta_abs - 90)
        sqterm = pool.tile([128, cc], F32, tag="sqterm")
        nc.scalar.activation(sqterm[:], au[:], AF.Sqrt, bias=1.0, scale=-1.0)
        x2 = pool.tile([128, cc], F32, tag="x2")
        nc.scalar.activation(x2[:], au[:], AF.Square)
        x4 = pool.tile([128, cc], F32, tag="x4")
        nc.scalar.activation(x4[:], x2[:], AF.Square)

        def pair(i_odd, col_even, tag):
            p = pool.tile([128, cc], F32, tag=tag)
            nc.vector.scalar_tensor_tensor(
                out=p[:], in0=au[:], scalar=float(ACOS_COEF[i_odd] * DEG),
                in1=coefs[:, col_even : col_even + 1].to_broadcast([128, cc]),
                op0=OP.mult, op1=OP.add,
            )
            return p

        p01 = pair(1, 0, "p01")
        p23 = pair(3, 2, "p23")
        p45 = pair(5, 4, "p45")
        p67 = pair(7, 6, "p67")
        t1 = pool.tile([128, cc], F32, tag="es1")
        nc.vector.tensor_tensor(out=t1[:], in0=x2[:], in1=p23[:], op=OP.mult)
        nc.vector.tensor_tensor(out=t1[:], in0=t1[:], in1=p01[:], op=OP.add)
        t2 = pool.tile([128, cc], F32, tag="es2")
        nc.vector.tensor_tensor(out=t2[:], in0=x2[:], in1=p67[:], op=OP.mult)
        nc.vector.tensor_tensor(out=t2[:], in0=t2[:], in1=p45[:], op=OP.add)
        nc.vector.tensor_tensor(out=t2[:], in0=t2[:], in1=x4[:], op=OP.mult)
        nc.vector.tensor_tensor(out=t1[:], in0=t1[:], in1=t2[:], op=OP.add)
        thabs = pool.tile([128, cc], F32, tag="thabs")
        nc.vector.tensor_tensor(out=thabs[:], in0=sqterm[:], in1=t1[:], op=OP.mult)
        theta = pool.tile([128, cc], F32, tag="theta")
        nc.vector.tensor_scalar(
            out=theta[:], in0=thabs[:], scalar1=-90.0, scalar2=None, op0=OP.add
        )
        nc.vector.tensor_tensor(out=theta[:], in0=theta[:], in1=sg[:], op=OP.mult)
        nc.vector.tensor_scalar(
            out=theta[:], in0=theta[:], scalar1=90.0, scalar2=None, op0=OP.add
        )
        if k == 0 and "dbg_theta" in outs:
            nc.sync.dma_start(out=outs["dbg_theta"][:], in_=theta[:])

        # fine bin q = round(theta/H) (convert rounding handled by probe: trunc)
        qf_pre = pool.tile([128, cc], F32, tag="qfpre")
        nc.vector.tensor_scalar(
            out=qf_pre[:], in0=theta[:], scalar1=1.0 / H, scalar2=0.5,
            op0=OP.mult, op1=OP.add,
        )
        q_i = pool.tile([128, cc], I32, tag="qi")
        nc.vector.tensor_copy(out=q_i[:], in_=qf_pre[:])
        qf = pool.tile([128, cc], F32, tag="qf")
        nc.vector.tensor_copy(out=qf[:], in_=q_i[:])
        eps = pool.tile([128, cc], F32, tag="eps")
        nc.vector.scalar_tensor_tensor(
            out=eps[:], in0=qf[:], scalar=-H, in1=theta[:], op0=OP.mult, op1=OP.add
        )
        qh_i = pool.tile([128, cc], I32, tag="qhi")
        nc.vector.tensor_scalar(
            out=qh_i[:], in0=q_i[:], scalar1=int(math.log2(QL)), scalar2=None,
            op0=OP.arith_shift_right
        )
        ql_i = pool.tile([128, cc], I32, tag="qli")
        nc.vector.tensor_scalar(
            out=ql_i[:], in0=q_i[:], scalar1=QL - 1, scalar2=None, op0=OP.bitwise_and
        )

        # moment payload E = mask * (1, eps, eps^2, eps^3)
        ee = pool.tile([128, cc, PMOM], F32, tag="ee")
        if mask is None:
            nc.vector.memset(ee[:, :, 0], 1.0)
            nc.vector.tensor_copy(out=ee[:, :, 1], in_=eps[:])
        else:
            nc.vector.tensor_copy(out=ee[:, :, 0], in_=mask[:])
            nc.vector.tensor_tensor(
                out=ee[:, :, 1], in0=eps[:], in1=mask[:], op=OP.mult
            )
        nc.vector.tensor_tensor(
            out=ee[:, :, 2], in0=ee[:, :, 1], in1=eps[:], op=OP.mult
        )
        nc.vector.tensor_tensor(
            out=ee[:, :, 3], in0=ee[:, :, 2], in1=eps[:], op=OP.mult
        )

        # one-hots
        oh_ql = pool.tile([128, cc, QL], F32, tag="ohql")
        nc.vector.tensor_tensor(
            out=oh_ql[:],
            in0=ql_i[:].unsqueeze(2).to_broadcast([128, cc, QL]),
            in1=iota_ql[:].unsqueeze(1).to_broadcast([128, cc, QL]),
            op=OP.is_equal,
        )
        oh_qh = pool.tile([128, cc, QH], F32, tag="ohqh")
        nc.vector.tensor_tensor(
            out=oh_qh[:],
            in0=qh_i[:].unsqueeze(2).to_broadcast([128, cc, QH]),
            in1=iota_qh[:].unsqueeze(1).to_broadcast([128, cc, QH]),
            op=OP.is_equal,
        )
        # lhsT[m, (ql, pm)] = oh_ql[m, ql] * E[m, pm]
        lhs = pool.tile([128, cc, QL * PMOM], F32, tag="lhs")
        nc.vector.tensor_tensor(
            out=lhs[:],
            in0=oh_ql[:].unsqueeze(3).to_broadcast([128, cc, QL, PMOM]),
            in1=ee[:].unsqueeze(2).to_broadcast([128, cc, QL, PMOM]),
            op=OP.mult,
        )

        for j in range(cc):
            nc.tensor.matmul(
                out=psum_m[:],
                lhsT=lhs[:, j, :],
                rhs=oh_qh[:, j, :],
                start=(k == 0 and j == 0),
                stop=(k == chunks - 1 and j == cc - 1),
            )

    # ---- moments out ----
    m_sb = const_pool.tile([QL * PMOM, QH], F32)
    nc.vector.tensor_copy(out=m_sb[:], in_=psum_m[:])
    if host_reduce:
        # ship the tiny per-core moment block; reduce + reconstruct on host
        nc.sync.dma_start(out=outs["outm"][:], in_=m_sb[:])
        return
    m_local = dram_pool.tile([QL * PMOM, QH], F32)
    nc.sync.dma_start(out=m_local[:], in_=m_sb[:])
    m_red = dram_pool.tile([QL * PMOM, QH], F32)
    nc.gpsimd.collective_compute(
        "AllReduce",
        OP.add,
        replica_groups=[list(range(N_CORES))],
        ins=[m_local[:].opt()],
        outs=[m_red[:].opt()],
    )
    # reload flat: element kk = p*QH + n ; rhs chunks [128, 6]
    m_rhs = const_pool.tile([128, 6], F32)
    nc.sync.dma_start(
        out=m_rhs[:], in_=m_red[:].rearrange("p n -> (p n)").rearrange("(c p) -> p c", p=128)
    )

    # ---- final contraction count[b] = sum_k M[k] * A[k, b] ----
    psum_ca = psum_pool.tile([128, 1], F32)
    psum_cb = psum_pool.tile([NBINS - 128, 1], F32)
    for cquad in range(6):
        nc.tensor.matmul(
            out=psum_ca[:], lhsT=a_sb[:, cquad, 0:128], rhs=m_rhs[:, cquad : cquad + 1],
            start=(cquad == 0), stop=(cquad == 5),
        )
    for cquad in range(6):
        nc.tensor.matmul(
            out=psum_cb[:], lhsT=a_sb[:, cquad, 128:NBINS], rhs=m_rhs[:, cquad : cquad + 1],
            start=(cquad == 0), stop=(cquad == 5),
        )
    cnt = const_pool.tile([128, 2], F32)
    nc.vector.memset(cnt[:], 0.0)
    nc.vector.tensor_copy(out=cnt[:, 0:1], in_=psum_ca[:])
    nc.vector.tensor_copy(out=cnt[0 : NBINS - 128, 1:2], in_=psum_cb[:])

    # total + normalize
    psum_t = psum_pool.tile([1, 2], F32)
    nc.tensor.matmul(out=psum_t[:], lhsT=ones_col[:], rhs=cnt[:], start=True, stop=True)
    tt = const_pool.tile([1, 2], F32)
    nc.vector.tensor_copy(out=tt[:], in_=psum_t[:])
    tot = const_pool.tile([1, 1], F32)
    nc.vector.tensor_tensor(out=tot[:], in0=tt[:, 0:1], in1=tt[:, 1:2], op=OP.add)
    rtot = const_pool.tile([1, 1], F32)
    nc.vector.reciprocal(rtot[:], tot[:])
    psum_r = psum_pool.tile([128, 1], F32)
    nc.tensor.matmul(out=psum_r[:], lhsT=ones_row[:], rhs=rtot[:], start=True, stop=True)
    outn = const_pool.tile([128, 2], F32)
    nc.vector.tensor_tensor(
        out=outn[:], in0=cnt[:], in1=psum_r[:].to_broadcast([128, 2]), op=OP.mult
    )
    nc.sync.dma_start(out=out[0:128], in_=outn[:, 0])
    nc.sync.dma_start(out=out[128:NBINS], in_=outn[0 : NBINS - 128, 1])


# ---------------- host side ----------------

def prep_global_inputs(xyz: np.ndarray, angle_list: np.ndarray):
    """Pack host-side directly into the core-concatenated global arrays."""
    flat = np.asarray(xyz, dtype=np.float32).reshape(-1, 3)  # [4096, 3]
    xyz3 = np.ascontiguousarray(flat.T)  # [3, 4096]
    al = np.asarray(angle_list)
    if al.dtype == np.int64:
        al = al.view(np.int32)[:, ::2]  # little-endian low words, zero-copy
    elif al.dtype != np.int32:
        al = al.astype(np.int32)
    per = angle_list.shape[0] // N_CORES
    assert per == PER_CORE and per <= SLOTS
    # one int32 per angle: f | a1<<3 | c<<12 | a2<<21. The concatenated
    # global array IS v_all (j-minor slot order s' = (k*128 + p)*C + j per
    # core); the device zero-fills the ragged last chunk, so no host padding
    # or copies are needed.
    v_all = al[:, 0] | (al[:, 1] << 3) | (al[:, 2] << 12) | (al[:, 3] << 21)
    return {"idx": np.ascontiguousarray(v_all, dtype=np.int32), "xyz3": xyz3}


_PROG_CACHE = {}


def build_program(chunks=CHUNKS, cols=C):
    key = (chunks, cols, _HOST_REDUCE)
    if key in _PROG_CACHE:
        return _PROG_CACHE[key]
    nc = bacc.Bacc("TRN2", target_bir_lowering=False, num_devices=N_CORES)
    idx_len = PER_CORE if chunks == CHUNKS else chunks * 128 * cols
    ins = {
        "idx": nc.dram_tensor("idx", [idx_len], I32, kind="ExternalInput").ap(),
        "xyz3": nc.dram_tensor("xyz3", [3, TBL], F32, kind="ExternalInput").ap(),
    }
    if _HOST_REDUCE:
        outs = {
            "outm": nc.dram_tensor(
                "outm", [QL * PMOM, QH], F32, kind="ExternalOutput"
            ).ap()
        }
    else:
        outs = {"out": nc.dram_tensor("out", [NBINS], F32, kind="ExternalOutput").ap()}
    # raw ap_gather buffers: must be allocated BEFORE TileContext so the tile
    # pools (which claim the free SBUF region at entry) don't overlap them.
    xyzt_sb = nc.alloc_sbuf_tensor("xyzt_sb", [128, TBL], F32)
    idxs16_raw = [
        nc.alloc_sbuf_tensor(f"idxs16r{i}", [128, 3 * cols], I16)
        for i in range(2)
    ]
    gath_raw = [
        nc.alloc_sbuf_tensor(f"gathr{i}", [128, 3 * 16 * cols], F32)
        for i in range(2)
    ]
    raw = (xyzt_sb, idxs16_raw, gath_raw)
    with tile.TileContext(nc) as tc:
        adf_kernel(tc, outs, ins, raw, per=PER_CORE if chunks == CHUNKS else None)
    nc.compile()
    _PROG_CACHE[key] = nc
    return nc


# ---------------- cached PJRT runner ----------------
# run_bass_kernel_spmd rebuilds its jax.jit closure on every call (full
# retrace + lowering each time). Build the jitted sharded callable ONCE and
# reuse it; each call still ships the packed inputs and runs the device.

_RUNNER = None


def _get_runner():
    global _RUNNER
    if _RUNNER is not None:
        return _RUNNER
    import jax
    from jax.sharding import Mesh, PartitionSpec
    from jax.experimental.shard_map import shard_map
    from concourse.bass2jax import (
        _bass_exec_p, install_neuronx_cc_hook, partition_id_tensor,
    )

    nc = build_program()
    install_neuronx_cc_hook()

    partition_name = nc.partition_id_tensor.name if nc.partition_id_tensor else None
    in_names, out_names, out_avals = [], [], []
    for alloc in nc.m.functions[0].allocations:
        if not isinstance(alloc, mybir.MemoryLocationSet):
            continue
        name = alloc.memorylocations[0].name
        if alloc.kind == "ExternalInput":
            if name != partition_name:
                in_names.append(name)
        elif alloc.kind == "ExternalOutput":
            shape = tuple(alloc.tensor_shape)
            dtype = mybir.dt.np(alloc.dtype)
            out_names.append(name)
            out_avals.append(jax.core.ShapedArray(shape, dtype))
    assert nc.dbg_addr is None, "debug build unsupported in cached PJRT runner"
    # the NKI lowering only consumes ExternalInput-named operands and the
    # alias map is empty, so no donated zero-output operands are needed; the
    # kernel writes every element of 'out'.
    in_names_full = in_names + (
        [partition_name] if partition_name is not None else []
    )

    def _body(*args):
        operands = list(args)
        if partition_name is not None:
            operands.append(partition_id_tensor())
        outs_ = _bass_exec_p.bind(
            *operands,
            out_avals=tuple(out_avals),
            in_names=tuple(in_names_full),
            out_names=tuple(out_names),
            lowering_input_output_aliases=(),
            sim_require_finite=True,
            sim_require_nnan=True,
            nc=nc,
        )
        return tuple(outs_)

    devices = jax.devices()[:N_CORES]
    assert len(devices) == N_CORES
    mesh = Mesh(np.asarray(devices), ("core",))
    # xyz3 is identical on every core: ship one copy, replicated in_spec
    in_specs = tuple(
        PartitionSpec() if n == "xyz3" else PartitionSpec("core") for n in in_names
    )
    sharded = jax.jit(
        shard_map(
            _body, mesh=mesh,
            in_specs=in_specs,
            out_specs=(PartitionSpec("core"),) * len(out_names),
            check_rep=False,
        ),
        keep_unused=True,
    )

    out_pos = {name: i for i, name in enumerate(out_names)}

    def run(global_map):
        concat_in = [global_map[name] for name in in_names]
        out_arrs = sharded(*concat_in)
        if "outm" in out_pos:
            # per-core moment blocks: fetch all 8 shards, reduce on host
            return np.asarray(out_arrs[out_pos["outm"]])
        # fetch only core 0's shard of 'out' (all cores produce identical output)
        shard = out_arrs[out_pos["out"]].addressable_shards[0].data
        return np.asarray(shard)

    _RUNNER = run
    return run


_AMAT = None


def kernel(**inputs) -> np.ndarray:
    global _AMAT
    xyz = np.asarray(inputs["xyz"], dtype=np.float32)
    angle_list = np.asarray(inputs["angle_list"])
    run = _get_runner()  # one-time program build + jit construction
    if _HOST_REDUCE and _AMAT is None:
        _AMAT = build_amat().astype(np.float64)
    # timed region matches the baseline definition: device dispatch + transfer
    # + exec + result fetch (host packing excluded, as in the original)
    gm = prep_global_inputs(xyz, angle_list)
    t0 = time.time()
    out = run(gm)
    if _HOST_REDUCE:
        m = out.reshape(N_CORES, QL * PMOM * QH).sum(axis=0, dtype=np.float64)
        count = m @ _AMAT
        out = (count / count.sum()).astype(np.float32)
    kernel._last_run_s = time.time() - t0
    kernel._last_results = None
    return np.asarray(out, dtype=np.float32)


if __name__ == "__main__":
    # smoke: build only
    build_program()
    print("program built ok")
